# revision 12
# baseline (speedup 1.0000x reference)
"""Trainium2 Bass kernel for fused multi-head attention block.

Per batch element b (one NeuronCore per element, 8 cores, pure data
parallelism, no collectives):

  q = x @ wq + bq ; k = x @ wk + bk ; v = x @ wv + bv      (16 heads x 64)
  scores = q k^T / sqrt(64) - 10000 * mask[k]
  attn   = softmax(scores)          (no max-subtraction; masked cols -> exact 0)
  ctx    = attn @ v
  out    = LayerNorm(x + ctx @ wo + bo) * gamma + beta

Everything is computed in the transposed orientation [k, q] so a single
exp pass feeds both the ctx matmul and the attention-probability output:
  - scores^T via a K=65 matmul (the mask bias rides as a 65th contraction
    row: kT row 64 = -80000*mask, qT row 64 = ones),
  - softmax denominators come free as row 64 of the ctx psum (a ones
    column appended to each v tile),
  - the reciprocal is broadcast across partitions via a tiny DRAM bounce,
  - attn is written to DRAM transposed; the host unshard step transposes
    it back (layout only, no FLOPs).
Host pre-transposes x and pre-casts weights to bf16 (marshalling only).
"""

import os
import sys

import numpy as np

for _p in ("/opt/trn_rl_repo",):
    if _p not in sys.path:
        sys.path.insert(0, _p)

import concourse.bass as bass  # noqa: E402
import concourse.tile as tile  # noqa: E402
from concourse import mybir  # noqa: E402
from concourse.bacc import Bacc  # noqa: E402
from concourse.bass_utils import run_bass_kernel_spmd  # noqa: E402

B, S, H, HS = 8, 1024, 16, 64
D = H * HS
KT = D // 128          # contraction tiles of 128
ST = S // 128          # sequence tiles of 128
PAIRS = H // 2
EPS = 1e-6
NINF_BIAS = -80000.0   # exp(0.125*(qk + bias_row)) == exp(qk/8 - 10000*mask)

F32 = mybir.dt.float32
BF16 = mybir.dt.bfloat16

_CACHE = {}


def _build(skip_bias=False, skip_gamma=False, skip_bo2=False):
    nc = Bacc("TRN2", target_bir_lowering=False, debug=False, enable_asserts=False)

    x_d = nc.dram_tensor("x", [S, D], F32, kind="ExternalInput")
    xT_d = nc.dram_tensor("xT", [D, S], BF16, kind="ExternalInput")
    bias8_d = nc.dram_tensor("bias8", [1, S], F32, kind="ExternalInput")
    wq_d = nc.dram_tensor("wq", [D, D], BF16, kind="ExternalInput")
    wk_d = nc.dram_tensor("wk", [D, D], BF16, kind="ExternalInput")
    wv_d = nc.dram_tensor("wv", [D, D], BF16, kind="ExternalInput")
    wo_d = nc.dram_tensor("wo", [D, D], BF16, kind="ExternalInput")
    bq2_d = nc.dram_tensor("bq2", [128, KT], F32, kind="ExternalInput")
    bk2_d = nc.dram_tensor("bk2", [128, KT], F32, kind="ExternalInput")
    bo2_d = nc.dram_tensor("bo2", [1, D], F32, kind="ExternalInput")
    gamma_d = nc.dram_tensor("gamma_r", [1, D], F32, kind="ExternalInput")
    beta_d = nc.dram_tensor("beta_r", [1, D], F32, kind="ExternalInput")

    out_d = nc.dram_tensor("out", [S, D], F32, kind="ExternalOutput")
    # attn stored TRANSPOSED per head: attn_d[h, k, q]; host swaps back.
    attn_d = nc.dram_tensor("attn", [H, S, S], BF16, kind="ExternalOutput")
    r32_d = nc.dram_tensor("r32_scratch", [H, S], F32)
    r16_d = nc.dram_tensor("r16_scratch", [H, S], BF16)

    idn_d = nc.inline_tensor(np.eye(128, dtype=np.float32), name="idn")

    Exp = mybir.ActivationFunctionType.Exp
    SqrtF = mybir.ActivationFunctionType.Sqrt
    AluAdd = mybir.AluOpType.add

    with tile.TileContext(nc) as tc:
        from contextlib import ExitStack

        with ExitStack() as ctx:
            persist = ctx.enter_context(tc.tile_pool(name="persist", bufs=1))
            wpool = ctx.enter_context(tc.tile_pool(name="w", bufs=24))
            qpool = ctx.enter_context(tc.tile_pool(name="qp", bufs=4))
            kpool = ctx.enter_context(tc.tile_pool(name="kp", bufs=4))
            ptpool = ctx.enter_context(tc.tile_pool(name="pb", bufs=12))
            srowpool = ctx.enter_context(tc.tile_pool(name="srow", bufs=2))
            recpool = ctx.enter_context(tc.tile_pool(name="rec", bufs=2))
            rtpool = ctx.enter_context(tc.tile_pool(name="rt", bufs=4))
            rbpool = ctx.enter_context(tc.tile_pool(name="rb", bufs=2))
            rb16pool = ctx.enter_context(tc.tile_pool(name="rb16", bufs=2))
            pspool = ctx.enter_context(tc.tile_pool(name="ps", bufs=2, space="PSUM"))
            cxpool = ctx.enter_context(tc.tile_pool(name="cx", bufs=4, space="PSUM"))

            # ---- persistent small tiles ----
            idn_sb = persist.tile([128, 128], F32, tag="idn")
            nc.sync.dma_start(idn_sb, idn_d[:, :])
            bq2_sb = persist.tile([128, KT], F32, tag="bq2")
            nc.sync.dma_start(bq2_sb, bq2_d[:, :])
            bk2_sb = persist.tile([128, KT], F32, tag="bk2")
            nc.sync.dma_start(bk2_sb, bk2_d[:, :])
            bo2_sb = persist.tile([1, D], BF16, tag="bo2")
            nc.gpsimd.dma_start(out=bo2_sb, in_=bo2_d[:, :])  # f32 -> bf16 cast
            gamma_b = persist.tile([128, D], F32, tag="gamma_b")
            nc.sync.dma_start(gamma_b, gamma_d[0:1, :].to_broadcast([128, D]))
            beta_b = persist.tile([128, D], F32, tag="beta_b")
            nc.sync.dma_start(beta_b, beta_d[0:1, :].to_broadcast([128, D]))
            ones1 = persist.tile([1, 128], BF16, tag="ones1")
            nc.gpsimd.memset(ones1, 1.0)
            eps_sb = persist.tile([128, 1], F32, tag="eps")
            nc.vector.memset(eps_sb, EPS)

            # ---- persistent big tiles ----
            xT = [persist.tile([128, S], BF16, tag=f"xT{j}", name=f"xT{j}")
                  for j in range(KT)]
            for j in range(KT):
                nc.sync.dma_start(xT[j], xT_d[j * 128:(j + 1) * 128, :])
            # v with a ones column per head: [128, 16*(64+1)]
            vE = [persist.tile([128, H * 65], BF16, tag=f"vE{t}", name=f"vE{t}")
                  for t in range(KT)]
            for t in range(KT):
                nc.gpsimd.memset(
                    vE[t].rearrange("p (h x) -> p h x", x=65)[:, :, 64:65], 1.0
                )
            ctxT = [persist.tile([128, S], BF16, tag=f"ctxT{t}", name=f"ctxT{t}")
                    for t in range(PAIRS)]

            # ---- weights (bf16, resident; wo reuses wv slots) ----
            w_q = [wpool.tile([128, D], BF16, tag="w", name=f"wq{i}") for i in range(KT)]
            w_k = [wpool.tile([128, D], BF16, tag="w", name=f"wk{i}") for i in range(KT)]
            w_v = [wpool.tile([128, D], BF16, tag="w", name=f"wv{i}") for i in range(KT)]
            for kt in range(KT):
                nc.sync.dma_start(w_q[kt], wq_d[kt * 128:(kt + 1) * 128, :])
                nc.sync.dma_start(w_k[kt], wk_d[kt * 128:(kt + 1) * 128, :])
                nc.sync.dma_start(w_v[kt], wv_d[kt * 128:(kt + 1) * 128, :])

            # ================= v projection (natural [s, d]) =================
            for t in range(ST):
                for dh in range(2):
                    ps = pspool.tile([128, 512], F32, tag="sc", name=f"vps{t}_{dh}")
                    for kt in range(KT):
                        nc.tensor.matmul(
                            ps,
                            lhsT=xT[kt][:, t * 128:(t + 1) * 128],
                            rhs=w_v[kt][:, dh * 512:(dh + 1) * 512],
                            start=(kt == 0),
                            stop=(kt == KT - 1),
                        )
                    nc.vector.tensor_copy(
                        vE[t][:, dh * 520:(dh + 1) * 520]
                        .rearrange("p (h x) -> p h x", x=65)[:, :, 0:64],
                        ps.rearrange("p (h x) -> p h x", x=64),
                    )

            w_o = [wpool.tile([128, D], BF16, tag="w", name=f"wo{i}") for i in range(KT)]
            for kt in range(KT):
                nc.sync.dma_start(w_o[kt], wo_d[kt * 128:(kt + 1) * 128, :])

            # ======== per-pair: q/k projection then attention (2 heads) ======
            for pair in range(PAIRS):
                heads = (2 * pair, 2 * pair + 1)
                qT2, kT2 = {}, {}
                for h in heads:
                    qT2[h] = qpool.tile([65, S], BF16, tag="qTe", name=f"qTe{h}")
                    nc.gpsimd.memset(qT2[h][64:65, :], 1.0)
                    kT2[h] = kpool.tile([65, S], BF16, tag="kTe", name=f"kTe{h}")
                    nc.gpsimd.dma_start(out=kT2[h][64:65, :], in_=bias8_d[0:1, :])

                for wi, (w_tiles, b_sb, dest) in enumerate(
                        ((w_q, bq2_sb, qT2), (w_k, bk2_sb, kT2))):
                    for sh in range(2):
                        ps = pspool.tile([128, 512], F32, tag="sc",
                                         name=f"qkps{pair}_{sh}")
                        for kt in range(KT):
                            nc.tensor.matmul(
                                ps,
                                lhsT=w_tiles[kt][:, pair * 128:(pair + 1) * 128],
                                rhs=xT[kt][:, sh * 512:(sh + 1) * 512],
                                start=(kt == 0),
                                stop=(kt == KT - 1),
                            )
                        for half in range(2):
                            h = 2 * pair + half
                            dst = dest[h][0:64, sh * 512:(sh + 1) * 512]
                            srcp = ps[half * 64:(half + 1) * 64, :]
                            if skip_bias:
                                if wi == 0:
                                    nc.scalar.copy(dst, srcp)
                                else:
                                    nc.vector.tensor_copy(dst, srcp)
                            elif wi == 0:
                                nc.scalar.activation(
                                    dst, srcp,
                                    mybir.ActivationFunctionType.Identity,
                                    bias=b_sb[half * 64:(half + 1) * 64,
                                              pair:pair + 1],
                                )
                            else:
                                nc.vector.tensor_scalar(
                                    dst, srcp,
                                    b_sb[half * 64:(half + 1) * 64, pair:pair + 1],
                                    None, op0=AluAdd,
                                )

                for h in heads:
                    qT, kT_ = qT2[h], kT2[h]
                    # --- scores^T [k, q] -> single exp -> pT (unnormalized) ---
                    pb = []
                    for kt in range(ST):
                        ps = pspool.tile([128, S], F32, tag="sc", name=f"sB{h}_{kt}")
                        for qh in range(2):
                            nc.tensor.matmul(
                                ps[:, qh * 512:(qh + 1) * 512],
                                lhsT=kT_[:, kt * 128:(kt + 1) * 128],
                                rhs=qT[:, qh * 512:(qh + 1) * 512],
                                start=True,
                                stop=True,
                            )
                        pt = ptpool.tile([128, S], BF16, tag="pt", name=f"pt{h}_{kt}")
                        nc.scalar.activation(pt, ps, Exp, scale=0.125)
                        pb.append(pt)
                    # --- ctx^T (+ sums in row 64 via the v ones column) ---
                    ctxE = []
                    for qh in range(2):
                        pc = cxpool.tile([65, 512], F32, tag="cx",
                                         name=f"ctx{h}_{qh}")
                        for kt in range(ST):
                            nc.tensor.matmul(
                                pc,
                                lhsT=vE[kt][:, h * 65:(h + 1) * 65],
                                rhs=pb[kt][:, qh * 512:(qh + 1) * 512],
                                start=(kt == 0),
                                stop=(kt == ST - 1),
                            )
                        ctxE.append(pc)
                    # --- softmax denominators -> reciprocal, broadcast [q] ---
                    srow = srowpool.tile([1, S], F32, tag="srow", name=f"srow{h}")
                    for qh in range(2):
                        nc.vector.tensor_copy(
                            srow[0:1, qh * 512:(qh + 1) * 512], ctxE[qh][64:65, :]
                        )
                    tp = cxpool.tile([128, ST], F32, tag="cx", name=f"tps{h}")
                    for j in range(ST):
                        nc.tensor.transpose(
                            tp[:, j:j + 1],
                            srow[0:1, j * 128:(j + 1) * 128],
                            idn_sb[0:1, 0:1],
                        )
                    recs = recpool.tile([128, ST], F32, tag="recs", name=f"recs{h}")
                    nc.vector.reciprocal(recs, tp)
                    rtr = cxpool.tile([8, 128], F32, tag="cx", name=f"rtr{h}")
                    nc.tensor.transpose(rtr, recs, idn_sb)
                    rT32 = rtpool.tile([8, 128], F32, tag="rT32", name=f"rT32{h}")
                    nc.vector.tensor_copy(rT32, rtr)
                    rT16 = rtpool.tile([8, 128], BF16, tag="rT16", name=f"rT16{h}")
                    nc.vector.tensor_copy(rT16, rT32)
                    nc.sync.dma_start(
                        r32_d[h:h + 1, :].rearrange("a (b c) -> (a b) c", c=128), rT32
                    )
                    nc.sync.dma_start(
                        r16_d[h:h + 1, :].rearrange("a (b c) -> (a b) c", c=128), rT16
                    )
                    rb32 = rbpool.tile([64, S], F32, tag="rb32", name=f"rb32{h}")
                    nc.sync.dma_start(rb32, r32_d[h:h + 1, :].to_broadcast([64, S]))
                    rb16 = rb16pool.tile([128, S], BF16, tag="rb16", name=f"rb16{h}")
                    nc.sync.dma_start(rb16, r16_d[h:h + 1, :].to_broadcast([128, S]))
                    # --- ctx normalize during psum->sbuf move ---
                    half = h % 2
                    for qh in range(2):
                        nc.vector.tensor_mul(
                            ctxT[h // 2][half * 64:(half + 1) * 64,
                                         qh * 512:(qh + 1) * 512],
                            ctxE[qh][0:64, :],
                            rb32[:, qh * 512:(qh + 1) * 512],
                        )
                    # --- attn^T normalize (idle GpSimd) + store ---
                    for kt in range(ST):
                        eng = nc.vector if kt % 2 == 0 else nc.gpsimd
                        eng.tensor_mul(pb[kt], pb[kt], rb16)
                        nc.sync.dma_start(
                            attn_d[h, kt * 128:(kt + 1) * 128, :], pb[kt]
                        )

            # ============== out-proj + residual + layernorm ===================
            with tc.tile_pool(name="p3x", bufs=2) as x2pool, \
                 tc.tile_pool(name="p3z", bufs=2) as zpool, \
                 tc.tile_pool(name="p3st", bufs=4) as statpool:
                for t in range(ST):
                    x2 = x2pool.tile([128, D], F32, tag="x2", name=f"x2{t}")
                    nc.sync.dma_start(x2, x_d[t * 128:(t + 1) * 128, :])
                    z = zpool.tile([128, D], F32, tag="z", name=f"z{t}")
                    for dh in range(2):
                        ps = pspool.tile([128, 512], F32, tag="sc", name=f"y{t}_{dh}")
                        for ct in range(PAIRS):
                            nc.tensor.matmul(
                                ps,
                                lhsT=ctxT[ct][:, t * 128:(t + 1) * 128],
                                rhs=w_o[ct][:, dh * 512:(dh + 1) * 512],
                                start=(ct == 0),
                                stop=(skip_bo2 and ct == PAIRS - 1),
                            )
                        if not skip_bo2:
                            nc.tensor.matmul(
                                ps,
                                lhsT=ones1,
                                rhs=bo2_sb[0:1, dh * 512:(dh + 1) * 512],
                                start=False,
                                stop=True,
                            )
                        nc.vector.tensor_add(
                            z[:, dh * 512:(dh + 1) * 512],
                            x2[:, dh * 512:(dh + 1) * 512],
                            ps,
                        )
                    stats = statpool.tile([128, 2, 6], F32, tag="stats", name=f"st{t}")
                    for sg in range(2):
                        nc.vector.bn_stats(
                            stats[:, sg, :], z[:, sg * 512:(sg + 1) * 512]
                        )
                    mv = statpool.tile([128, 2], F32, tag="mv", name=f"mv{t}")
                    nc.vector.bn_aggr(mv, stats)
                    sd = statpool.tile([128, 1], F32, tag="sd", name=f"sd{t}")
                    nc.scalar.activation(sd, mv[:, 1:2], SqrtF, bias=eps_sb)
                    rs = statpool.tile([128, 1], F32, tag="rs", name=f"rs{t}")
                    nc.vector.reciprocal(rs, sd)
                    nc.vector.tensor_scalar(
                        z, z, mv[:, 0:1], rs,
                        op0=mybir.AluOpType.subtract,
                        op1=mybir.AluOpType.mult,
                    )
                    if not skip_gamma:
                        nc.vector.tensor_mul(z, z, gamma_b)
                        nc.vector.tensor_add(z, z, beta_b)
                    nc.sync.dma_start(out_d[t * 128:(t + 1) * 128, :], z)

    nc.compile()
    return nc


def _get_nc(skip_bias=False, skip_gamma=False, skip_bo2=False):
    key = ("nc", skip_bias, skip_gamma, skip_bo2)
    if key not in _CACHE:
        _CACHE[key] = _build(skip_bias, skip_gamma, skip_bo2)
    return _CACHE[key]


def _install_ntff_hook():
    """Provide the antenv.axon_hooks shim the boot image lacks, so
    run_bass_kernel_spmd(trace=True) can capture NTFF profiles."""
    try:
        import types

        try:
            from antenv.axon_hooks import get_axon_ntff_profile_hook  # noqa: F401
        except ImportError:
            import antenv

            mod = types.ModuleType("antenv.axon_hooks")
            _hook = [None]
            mod.set_axon_ntff_profile_hook = lambda h: _hook.__setitem__(0, h)
            mod.get_axon_ntff_profile_hook = lambda: _hook[0]
            sys.modules["antenv.axon_hooks"] = mod
            antenv.axon_hooks = mod
        from antenv import axon_hooks

        if axon_hooks.get_axon_ntff_profile_hook() is None:
            from trn_agent_boot.trn_boot import _ntff_profile_via_ctypes

            hook = _ntff_profile_via_ctypes("/opt/axon/libaxon_pjrt.so")
            if hook is None:
                return False
            axon_hooks.set_axon_ntff_profile_hook(hook)
        import concourse.bass_utils as bu

        bu.upload_artifacts = lambda tmpdir: f"local:{tmpdir}"
        return True
    except Exception:
        import traceback

        traceback.print_exc()
        return False


def _to_bf16(a):
    import ml_dtypes

    return np.asarray(a, np.float32).astype(ml_dtypes.bfloat16)


def kernel(x, mask, wq, bq, wk, bk, wv, bv, wo, bo, gamma, beta):
    x = np.asarray(x, np.float32)
    mask = np.asarray(mask)
    wq, wk, wv, wo = (np.asarray(w, np.float32) for w in (wq, wk, wv, wo))
    bq, bk, bv, bo = (np.asarray(b, np.float32) for b in (bq, bk, bv, bo))
    gamma = np.asarray(gamma, np.float32)
    beta = np.asarray(beta, np.float32)

    skip_bias = bool(np.all(bq == 0.0) and np.all(bk == 0.0))
    skip_gamma = bool(np.all(gamma == 1.0) and np.all(beta == 0.0))
    bo2_chk = bo.astype(np.float64) + bv.astype(np.float64) @ wo.astype(np.float64)
    skip_bo2 = bool(np.all(bo2_chk == 0.0))
    nc = _get_nc(skip_bias, skip_gamma, skip_bo2)

    bo2 = (bo.astype(np.float64) + bv.astype(np.float64) @ wo.astype(np.float64))
    bo2 = bo2.astype(np.float32).reshape(1, D)
    shared = {
        "wq": _to_bf16(wq), "wk": _to_bf16(wk),
        "wv": _to_bf16(wv), "wo": _to_bf16(wo),
        "bq2": np.ascontiguousarray(bq.reshape(KT, 128).T),
        "bk2": np.ascontiguousarray(bk.reshape(KT, 128).T),
        "bo2": bo2,
        "gamma_r": gamma.reshape(1, D),
        "beta_r": beta.reshape(1, D),
    }
    in_maps = []
    for c in range(B):
        m = dict(shared)
        m["x"] = np.ascontiguousarray(x[c])
        m["xT"] = np.ascontiguousarray(_to_bf16(x[c].T))
        m["bias8"] = (NINF_BIAS * mask[c].astype(np.float32)).reshape(1, S)
        in_maps.append(m)

    trace = bool(int(os.environ.get("KERNEL_TRACE", "0")))
    if trace:
        trace = _install_ntff_hook()
    try:
        res = run_bass_kernel_spmd(
            nc, in_maps, core_ids=list(range(B)), trace=trace,
        )
    except Exception:
        if not trace:
            raise
        import traceback

        traceback.print_exc()
        res = run_bass_kernel_spmd(
            nc, in_maps, core_ids=list(range(B)), trace=False,
        )
    _CACHE["last_result"] = res

    out = np.stack([np.asarray(res.results[c]["out"], np.float32) for c in range(B)])
    # attn comes back per-head transposed [H, k, q]; swap back to [H, q, k]
    attn = np.empty((B, H, S, S), np.float32)
    for c in range(B):
        attn[c] = np.asarray(res.results[c]["attn"]).astype(np.float32).swapaxes(1, 2)
    return out, attn


# revision 14
# speedup vs baseline: 1.2322x; 1.2322x over previous
"""Trainium2 Bass kernel for fused multi-head attention block.

Per batch element b (one NeuronCore per element, 8 cores, pure data
parallelism, no collectives):

  q = x @ wq + bq ; k = x @ wk + bk ; v = x @ wv + bv      (16 heads x 64)
  scores = q k^T / sqrt(64) - 10000 * mask[k]
  attn   = softmax(scores)          (no max-subtraction; masked cols -> exact 0)
  ctx    = attn @ v
  out    = LayerNorm(x + ctx @ wo + bo) * gamma + beta

Everything is computed in the transposed orientation [k, q] so a single
exp pass feeds both the ctx matmul and the attention-probability output:
  - scores^T via a K=65 matmul (the mask bias rides as a 65th contraction
    row: kT row 64 = -80000*mask, qT row 64 = ones),
  - softmax denominators come free as row 64 of the ctx psum (a ones
    column appended to each v tile),
  - the reciprocal is broadcast across partitions via a tiny DRAM bounce,
  - attn is written to DRAM transposed; the host unshard step transposes
    it back (layout only, no FLOPs).
Host pre-transposes x and pre-casts weights to bf16 (marshalling only).
"""

import os
import sys

import numpy as np

for _p in ("/opt/trn_rl_repo",):
    if _p not in sys.path:
        sys.path.insert(0, _p)

import concourse.bass as bass  # noqa: E402
import concourse.tile as tile  # noqa: E402
from concourse import mybir  # noqa: E402
from concourse.bacc import Bacc  # noqa: E402
from concourse.bass_utils import run_bass_kernel_spmd  # noqa: E402

B, S, H, HS = 8, 1024, 16, 64
D = H * HS
KT = D // 128          # contraction tiles of 128
ST = S // 128          # sequence tiles of 128
PAIRS = H // 2
EPS = 1e-6
NINF_BIAS = -80000.0   # exp(0.125*(qk + bias_row)) == exp(qk/8 - 10000*mask)

F32 = mybir.dt.float32
BF16 = mybir.dt.bfloat16

_CACHE = {}


def _build(skip_bias=False, skip_gamma=False, skip_bo2=False):
    nc = Bacc("TRN2", target_bir_lowering=False, debug=False, enable_asserts=False)

    x_d = nc.dram_tensor("x", [S, D], F32, kind="ExternalInput")
    xT_d = nc.dram_tensor("xT", [D, S], BF16, kind="ExternalInput")
    bias8_d = nc.dram_tensor("bias8", [1, S], F32, kind="ExternalInput")
    wq_d = nc.dram_tensor("wq", [D, D], BF16, kind="ExternalInput")
    wk_d = nc.dram_tensor("wk", [D, D], BF16, kind="ExternalInput")
    wv_d = nc.dram_tensor("wv", [D, D], BF16, kind="ExternalInput")
    wo_d = nc.dram_tensor("wo", [D, D], BF16, kind="ExternalInput")
    bq2_d = nc.dram_tensor("bq2", [128, KT], F32, kind="ExternalInput")
    bk2_d = nc.dram_tensor("bk2", [128, KT], F32, kind="ExternalInput")
    bo2_d = nc.dram_tensor("bo2", [1, D], F32, kind="ExternalInput")
    gamma_d = nc.dram_tensor("gamma_r", [1, D], F32, kind="ExternalInput")
    beta_d = nc.dram_tensor("beta_r", [1, D], F32, kind="ExternalInput")

    out_d = nc.dram_tensor("out", [S, D], F32, kind="ExternalOutput")
    # attn stored TRANSPOSED per head: attn_d[h, k, q]; host swaps back.
    attn_d = nc.dram_tensor("attn", [H, S, S], BF16, kind="ExternalOutput")
    r16_d = nc.dram_tensor("r16_scratch", [H, S], BF16)

    idn_d = nc.inline_tensor(np.eye(128, dtype=np.float32), name="idn")

    Exp = mybir.ActivationFunctionType.Exp
    SqrtF = mybir.ActivationFunctionType.Sqrt
    AluAdd = mybir.AluOpType.add

    with tile.TileContext(nc) as tc:
        from contextlib import ExitStack

        with ExitStack() as ctx:
            persist = ctx.enter_context(tc.tile_pool(name="persist", bufs=1))
            wpool = ctx.enter_context(tc.tile_pool(name="w", bufs=24))
            qpool = ctx.enter_context(tc.tile_pool(name="qp", bufs=4))
            kpool = ctx.enter_context(tc.tile_pool(name="kp", bufs=4))
            ptpool = ctx.enter_context(tc.tile_pool(name="pb", bufs=14))
            srowpool = ctx.enter_context(tc.tile_pool(name="srow", bufs=2))
            recpool = ctx.enter_context(tc.tile_pool(name="rec", bufs=2))
            rtpool = ctx.enter_context(tc.tile_pool(name="rt", bufs=4))
            rb16pool = ctx.enter_context(tc.tile_pool(name="rb16", bufs=3))
            pspool = ctx.enter_context(tc.tile_pool(name="ps", bufs=2, space="PSUM"))
            cxpool = ctx.enter_context(tc.tile_pool(name="cx", bufs=4, space="PSUM"))

            # ---- persistent small tiles ----
            idn_sb = persist.tile([128, 128], F32, tag="idn")
            nc.sync.dma_start(idn_sb, idn_d[:, :])
            bq2_sb = persist.tile([128, KT], F32, tag="bq2")
            nc.sync.dma_start(bq2_sb, bq2_d[:, :])
            bk2_sb = persist.tile([128, KT], F32, tag="bk2")
            nc.sync.dma_start(bk2_sb, bk2_d[:, :])
            bo2_sb = persist.tile([1, D], BF16, tag="bo2")
            nc.gpsimd.dma_start(out=bo2_sb, in_=bo2_d[:, :])  # f32 -> bf16 cast
            gamma_b = persist.tile([128, D], F32, tag="gamma_b")
            nc.sync.dma_start(gamma_b, gamma_d[0:1, :].to_broadcast([128, D]))
            beta_b = persist.tile([128, D], F32, tag="beta_b")
            nc.sync.dma_start(beta_b, beta_d[0:1, :].to_broadcast([128, D]))
            ones1 = persist.tile([1, 128], BF16, tag="ones1")
            nc.gpsimd.memset(ones1, 1.0)
            eps_sb = persist.tile([128, 1], F32, tag="eps")
            nc.vector.memset(eps_sb, EPS)

            # ---- persistent big tiles ----
            xT = [persist.tile([128, S], BF16, tag=f"xT{j}", name=f"xT{j}")
                  for j in range(KT)]
            for j in range(KT):
                nc.sync.dma_start(xT[j], xT_d[j * 128:(j + 1) * 128, :])
            # v with a ones column per head: [128, 16*(64+1)]
            vE = [persist.tile([128, H * 65], BF16, tag=f"vE{t}", name=f"vE{t}")
                  for t in range(KT)]
            for t in range(KT):
                nc.gpsimd.memset(
                    vE[t].rearrange("p (h x) -> p h x", x=65)[:, :, 64:65], 1.0
                )
            ctxT = [persist.tile([128, S], BF16, tag=f"ctxT{t}", name=f"ctxT{t}")
                    for t in range(PAIRS)]

            # ---- weights (bf16, resident; wo reuses wv slots) ----
            w_q = [wpool.tile([128, D], BF16, tag="w", name=f"wq{i}") for i in range(KT)]
            w_k = [wpool.tile([128, D], BF16, tag="w", name=f"wk{i}") for i in range(KT)]
            w_v = [wpool.tile([128, D], BF16, tag="w", name=f"wv{i}") for i in range(KT)]
            for kt in range(KT):
                nc.sync.dma_start(w_q[kt], wq_d[kt * 128:(kt + 1) * 128, :])
                nc.sync.dma_start(w_k[kt], wk_d[kt * 128:(kt + 1) * 128, :])
                nc.sync.dma_start(w_v[kt], wv_d[kt * 128:(kt + 1) * 128, :])

            # ================= v projection (natural [s, d]) =================
            for t in range(ST):
                for dh in range(2):
                    ps = pspool.tile([128, 512], F32, tag="sc", name=f"vps{t}_{dh}")
                    for kt in range(KT):
                        nc.tensor.matmul(
                            ps,
                            lhsT=xT[kt][:, t * 128:(t + 1) * 128],
                            rhs=w_v[kt][:, dh * 512:(dh + 1) * 512],
                            start=(kt == 0),
                            stop=(kt == KT - 1),
                        )
                    nc.vector.tensor_copy(
                        vE[t][:, dh * 520:(dh + 1) * 520]
                        .rearrange("p (h x) -> p h x", x=65)[:, :, 0:64],
                        ps.rearrange("p (h x) -> p h x", x=64),
                    )

            w_o = [wpool.tile([128, D], BF16, tag="w", name=f"wo{i}") for i in range(KT)]
            for kt in range(KT):
                nc.sync.dma_start(w_o[kt], wo_d[kt * 128:(kt + 1) * 128, :])

            # ======== per-pair: q/k projection then attention (2 heads) ======
            for pair in range(PAIRS):
                heads = (2 * pair, 2 * pair + 1)
                qT2, kT2 = {}, {}
                for h in heads:
                    qT2[h] = qpool.tile([65, S], BF16, tag="qTe", name=f"qTe{h}")
                    nc.gpsimd.memset(qT2[h][64:65, :], 1.0)
                    kT2[h] = kpool.tile([65, S], BF16, tag="kTe", name=f"kTe{h}")
                    nc.gpsimd.dma_start(out=kT2[h][64:65, :], in_=bias8_d[0:1, :])

                for wi, (w_tiles, b_sb, dest) in enumerate(
                        ((w_q, bq2_sb, qT2), (w_k, bk2_sb, kT2))):
                    for sh in range(2):
                        ps = pspool.tile([128, 512], F32, tag="sc",
                                         name=f"qkps{pair}_{sh}")
                        for kt in range(KT):
                            nc.tensor.matmul(
                                ps,
                                lhsT=w_tiles[kt][:, pair * 128:(pair + 1) * 128],
                                rhs=xT[kt][:, sh * 512:(sh + 1) * 512],
                                start=(kt == 0),
                                stop=(kt == KT - 1),
                            )
                        for half in range(2):
                            h = 2 * pair + half
                            dst = dest[h][0:64, sh * 512:(sh + 1) * 512]
                            srcp = ps[half * 64:(half + 1) * 64, :]
                            if skip_bias:
                                nc.scalar.copy(dst, srcp)
                            else:
                                nc.scalar.activation(
                                    dst, srcp,
                                    mybir.ActivationFunctionType.Identity,
                                    bias=b_sb[half * 64:(half + 1) * 64,
                                              pair:pair + 1],
                                )

                for h in heads:
                    qT, kT_ = qT2[h], kT2[h]
                    # --- scores^T [k, q] -> single exp -> pT (unnormalized) ---
                    pb = []
                    for kt in range(ST):
                        ps = pspool.tile([128, S], F32, tag="sc", name=f"sB{h}_{kt}")
                        for qh in range(2):
                            nc.tensor.matmul(
                                ps[:, qh * 512:(qh + 1) * 512],
                                lhsT=kT_[:, kt * 128:(kt + 1) * 128],
                                rhs=qT[:, qh * 512:(qh + 1) * 512],
                                start=True,
                                stop=True,
                            )
                        pt = ptpool.tile([128, S], BF16, tag="pt", name=f"pt{h}_{kt}")
                        nc.scalar.activation(pt, ps, Exp, scale=0.125)
                        pb.append(pt)
                    # --- ctx^T (+ sums in row 64 via the v ones column) ---
                    ctxE = []
                    for qh in range(2):
                        pc = cxpool.tile([65, 512], F32, tag="cx",
                                         name=f"ctx{h}_{qh}")
                        for kt in range(ST):
                            nc.tensor.matmul(
                                pc,
                                lhsT=vE[kt][:, h * 65:(h + 1) * 65],
                                rhs=pb[kt][:, qh * 512:(qh + 1) * 512],
                                start=(kt == 0),
                                stop=(kt == ST - 1),
                            )
                        ctxE.append(pc)
                    # copy ctx out unnormalized; frees the psum slots fast
                    half = h % 2
                    for qh in range(2):
                        nc.vector.tensor_copy(
                            ctxT[h // 2][half * 64:(half + 1) * 64,
                                         qh * 512:(qh + 1) * 512],
                            ctxE[qh][0:64, :],
                        )
                    # --- softmax denominators -> reciprocal, broadcast [q] ---
                    srow = srowpool.tile([1, S], F32, tag="srow", name=f"srow{h}")
                    for qh in range(2):
                        nc.scalar.copy(
                            srow[0:1, qh * 512:(qh + 1) * 512], ctxE[qh][64:65, :]
                        )
                    tp = cxpool.tile([128, ST], F32, tag="cx", name=f"tps{h}")
                    for j in range(ST):
                        nc.tensor.transpose(
                            tp[:, j:j + 1],
                            srow[0:1, j * 128:(j + 1) * 128],
                            idn_sb[0:1, 0:1],
                        )
                    recs = recpool.tile([128, ST], F32, tag="recs", name=f"recs{h}")
                    nc.vector.reciprocal(recs, tp)
                    rtr = cxpool.tile([8, 128], F32, tag="cx", name=f"rtr{h}")
                    nc.tensor.transpose(rtr, recs, idn_sb)
                    rT16 = rtpool.tile([8, 128], BF16, tag="rT16", name=f"rT16{h}")
                    nc.vector.tensor_copy(rT16, rtr)
                    nc.sync.dma_start(
                        r16_d[h:h + 1, :].rearrange("a (b c) -> (a b) c", c=128), rT16
                    )
                    rb16 = rb16pool.tile([128, S], BF16, tag="rb16", name=f"rb16{h}")
                    nc.sync.dma_start(rb16, r16_d[h:h + 1, :].to_broadcast([128, S]))
                    # --- ctx normalize in place (sbuf, bf16) ---
                    for qh in range(2):
                        sl = ctxT[h // 2][half * 64:(half + 1) * 64,
                                          qh * 512:(qh + 1) * 512]
                        nc.vector.tensor_mul(
                            sl, sl,
                            rb16[half * 64:(half + 1) * 64,
                                 qh * 512:(qh + 1) * 512],
                        )
                    # --- attn^T normalize + store ---
                    for kt in range(ST):
                        nc.vector.tensor_mul(pb[kt], pb[kt], rb16)
                        nc.sync.dma_start(
                            attn_d[h, kt * 128:(kt + 1) * 128, :], pb[kt]
                        )

            # ============== out-proj + residual + layernorm ===================
            with tc.tile_pool(name="p3x", bufs=2) as x2pool, \
                 tc.tile_pool(name="p3z", bufs=2) as zpool, \
                 tc.tile_pool(name="p3st", bufs=4) as statpool:
                for t in range(ST):
                    x2 = x2pool.tile([128, D], F32, tag="x2", name=f"x2{t}")
                    nc.sync.dma_start(x2, x_d[t * 128:(t + 1) * 128, :])
                    z = zpool.tile([128, D], F32, tag="z", name=f"z{t}")
                    for dh in range(2):
                        ps = pspool.tile([128, 512], F32, tag="sc", name=f"y{t}_{dh}")
                        for ct in range(PAIRS):
                            nc.tensor.matmul(
                                ps,
                                lhsT=ctxT[ct][:, t * 128:(t + 1) * 128],
                                rhs=w_o[ct][:, dh * 512:(dh + 1) * 512],
                                start=(ct == 0),
                                stop=(skip_bo2 and ct == PAIRS - 1),
                            )
                        if not skip_bo2:
                            nc.tensor.matmul(
                                ps,
                                lhsT=ones1,
                                rhs=bo2_sb[0:1, dh * 512:(dh + 1) * 512],
                                start=False,
                                stop=True,
                            )
                        nc.vector.tensor_add(
                            z[:, dh * 512:(dh + 1) * 512],
                            x2[:, dh * 512:(dh + 1) * 512],
                            ps,
                        )
                    stats = statpool.tile([128, 2, 6], F32, tag="stats", name=f"st{t}")
                    for sg in range(2):
                        nc.vector.bn_stats(
                            stats[:, sg, :], z[:, sg * 512:(sg + 1) * 512]
                        )
                    mv = statpool.tile([128, 2], F32, tag="mv", name=f"mv{t}")
                    nc.vector.bn_aggr(mv, stats)
                    sd = statpool.tile([128, 1], F32, tag="sd", name=f"sd{t}")
                    nc.scalar.activation(sd, mv[:, 1:2], SqrtF, bias=eps_sb)
                    rs = statpool.tile([128, 1], F32, tag="rs", name=f"rs{t}")
                    nc.vector.reciprocal(rs, sd)
                    nc.vector.tensor_scalar(
                        z, z, mv[:, 0:1], rs,
                        op0=mybir.AluOpType.subtract,
                        op1=mybir.AluOpType.mult,
                    )
                    if not skip_gamma:
                        nc.vector.tensor_mul(z, z, gamma_b)
                        nc.vector.tensor_add(z, z, beta_b)
                    nc.sync.dma_start(out_d[t * 128:(t + 1) * 128, :], z)

    nc.compile()
    return nc


def _get_nc(skip_bias=False, skip_gamma=False, skip_bo2=False):
    key = ("nc", skip_bias, skip_gamma, skip_bo2)
    if key not in _CACHE:
        _CACHE[key] = _build(skip_bias, skip_gamma, skip_bo2)
    return _CACHE[key]


def _install_ntff_hook():
    """Provide the antenv.axon_hooks shim the boot image lacks, so
    run_bass_kernel_spmd(trace=True) can capture NTFF profiles."""
    try:
        import types

        try:
            from antenv.axon_hooks import get_axon_ntff_profile_hook  # noqa: F401
        except ImportError:
            import antenv

            mod = types.ModuleType("antenv.axon_hooks")
            _hook = [None]
            mod.set_axon_ntff_profile_hook = lambda h: _hook.__setitem__(0, h)
            mod.get_axon_ntff_profile_hook = lambda: _hook[0]
            sys.modules["antenv.axon_hooks"] = mod
            antenv.axon_hooks = mod
        from antenv import axon_hooks

        if axon_hooks.get_axon_ntff_profile_hook() is None:
            from trn_agent_boot.trn_boot import _ntff_profile_via_ctypes

            hook = _ntff_profile_via_ctypes("/opt/axon/libaxon_pjrt.so")
            if hook is None:
                return False
            axon_hooks.set_axon_ntff_profile_hook(hook)
        import concourse.bass_utils as bu

        bu.upload_artifacts = lambda tmpdir: f"local:{tmpdir}"
        return True
    except Exception:
        import traceback

        traceback.print_exc()
        return False


def _to_bf16(a):
    import ml_dtypes

    return np.asarray(a, np.float32).astype(ml_dtypes.bfloat16)


def kernel(x, mask, wq, bq, wk, bk, wv, bv, wo, bo, gamma, beta):
    x = np.asarray(x, np.float32)
    mask = np.asarray(mask)
    wq, wk, wv, wo = (np.asarray(w, np.float32) for w in (wq, wk, wv, wo))
    bq, bk, bv, bo = (np.asarray(b, np.float32) for b in (bq, bk, bv, bo))
    gamma = np.asarray(gamma, np.float32)
    beta = np.asarray(beta, np.float32)

    skip_bias = bool(np.all(bq == 0.0) and np.all(bk == 0.0))
    skip_gamma = bool(np.all(gamma == 1.0) and np.all(beta == 0.0))
    bo2_chk = bo.astype(np.float64) + bv.astype(np.float64) @ wo.astype(np.float64)
    skip_bo2 = bool(np.all(bo2_chk == 0.0))
    nc = _get_nc(skip_bias, skip_gamma, skip_bo2)

    bo2 = (bo.astype(np.float64) + bv.astype(np.float64) @ wo.astype(np.float64))
    bo2 = bo2.astype(np.float32).reshape(1, D)
    shared = {
        "wq": _to_bf16(wq), "wk": _to_bf16(wk),
        "wv": _to_bf16(wv), "wo": _to_bf16(wo),
        "bq2": np.ascontiguousarray(bq.reshape(KT, 128).T),
        "bk2": np.ascontiguousarray(bk.reshape(KT, 128).T),
        "bo2": bo2,
        "gamma_r": gamma.reshape(1, D),
        "beta_r": beta.reshape(1, D),
    }
    in_maps = []
    for c in range(B):
        m = dict(shared)
        m["x"] = np.ascontiguousarray(x[c])
        m["xT"] = np.ascontiguousarray(_to_bf16(x[c].T))
        m["bias8"] = (NINF_BIAS * mask[c].astype(np.float32)).reshape(1, S)
        in_maps.append(m)

    trace = bool(int(os.environ.get("KERNEL_TRACE", "0")))
    if trace:
        trace = _install_ntff_hook()
    try:
        res = run_bass_kernel_spmd(
            nc, in_maps, core_ids=list(range(B)), trace=trace,
        )
    except Exception:
        if not trace:
            raise
        import traceback

        traceback.print_exc()
        res = run_bass_kernel_spmd(
            nc, in_maps, core_ids=list(range(B)), trace=False,
        )
    _CACHE["last_result"] = res

    out = np.stack([np.asarray(res.results[c]["out"], np.float32) for c in range(B)])
    # attn comes back per-head transposed [H, k, q]; swap back to [H, q, k]
    attn = np.empty((B, H, S, S), np.float32)
    for c in range(B):
        attn[c] = np.asarray(res.results[c]["attn"]).astype(np.float32).swapaxes(1, 2)
    return out, attn


# revision 15
# speedup vs baseline: 1.2738x; 1.0338x over previous
"""Trainium2 Bass kernel for fused multi-head attention block.

Per batch element b (one NeuronCore per element, 8 cores, pure data
parallelism, no collectives):

  q = x @ wq + bq ; k = x @ wk + bk ; v = x @ wv + bv      (16 heads x 64)
  scores = q k^T / sqrt(64) - 10000 * mask[k]
  attn   = softmax(scores)          (no max-subtraction; masked cols -> exact 0)
  ctx    = attn @ v
  out    = LayerNorm(x + ctx @ wo + bo) * gamma + beta

Everything is computed in the transposed orientation [k, q] so a single
exp pass feeds both the ctx matmul and the attention-probability output:
  - scores^T via a K=65 matmul (the mask bias rides as a 65th contraction
    row: kT row 64 = -80000*mask, qT row 64 = ones),
  - softmax denominators come free as row 64 of the ctx psum (a ones
    column appended to each v tile),
  - the reciprocal is broadcast across partitions via a tiny DRAM bounce,
  - attn is written to DRAM transposed; the host unshard step transposes
    it back (layout only, no FLOPs).
Host pre-transposes x and pre-casts weights to bf16 (marshalling only).
"""

import os
import sys

import numpy as np

for _p in ("/opt/trn_rl_repo",):
    if _p not in sys.path:
        sys.path.insert(0, _p)

import concourse.bass as bass  # noqa: E402
import concourse.tile as tile  # noqa: E402
from concourse import mybir  # noqa: E402
from concourse.bacc import Bacc  # noqa: E402
from concourse.bass_utils import run_bass_kernel_spmd  # noqa: E402

B, S, H, HS = 8, 1024, 16, 64
D = H * HS
KT = D // 128          # contraction tiles of 128
ST = S // 128          # sequence tiles of 128
PAIRS = H // 2
EPS = 1e-6
NINF_BIAS = -80000.0   # exp(0.125*(qk + bias_row)) == exp(qk/8 - 10000*mask)

F32 = mybir.dt.float32
BF16 = mybir.dt.bfloat16

_CACHE = {}


def _build(skip_bias=False, skip_gamma=False, skip_bo2=False):
    nc = Bacc("TRN2", target_bir_lowering=False, debug=False, enable_asserts=False)

    x_d = nc.dram_tensor("x", [S, D], F32, kind="ExternalInput")
    xT_d = nc.dram_tensor("xT", [D, S], BF16, kind="ExternalInput")
    bias8_d = nc.dram_tensor("bias8", [1, S], F32, kind="ExternalInput")
    wq_d = nc.dram_tensor("wq", [D, D], BF16, kind="ExternalInput")
    wk_d = nc.dram_tensor("wk", [D, D], BF16, kind="ExternalInput")
    wv_d = nc.dram_tensor("wv", [D, D], BF16, kind="ExternalInput")
    wo_d = nc.dram_tensor("wo", [D, D], BF16, kind="ExternalInput")
    bq2_d = nc.dram_tensor("bq2", [128, KT], F32, kind="ExternalInput")
    bk2_d = nc.dram_tensor("bk2", [128, KT], F32, kind="ExternalInput")
    bo2_d = nc.dram_tensor("bo2", [1, D], F32, kind="ExternalInput")
    gamma_d = nc.dram_tensor("gamma_r", [1, D], F32, kind="ExternalInput")
    beta_d = nc.dram_tensor("beta_r", [1, D], F32, kind="ExternalInput")

    out_d = nc.dram_tensor("out", [S, D], F32, kind="ExternalOutput")
    # attn stored TRANSPOSED per head: attn_d[h, k, q]; host swaps back.
    attn_d = nc.dram_tensor("attn", [H, S, S], BF16, kind="ExternalOutput")
    r16_d = nc.dram_tensor("r16_scratch", [H, S], BF16)

    idn_d = nc.inline_tensor(np.eye(128, dtype=np.float32), name="idn")

    Exp = mybir.ActivationFunctionType.Exp
    SqrtF = mybir.ActivationFunctionType.Sqrt
    AluAdd = mybir.AluOpType.add

    with tile.TileContext(nc) as tc:
        from contextlib import ExitStack

        with ExitStack() as ctx:
            persist = ctx.enter_context(tc.tile_pool(name="persist", bufs=1))
            wpool = ctx.enter_context(tc.tile_pool(name="w", bufs=24))
            qpool = ctx.enter_context(tc.tile_pool(name="qp", bufs=4))
            kpool = ctx.enter_context(tc.tile_pool(name="kp", bufs=4))
            ptpool = ctx.enter_context(tc.tile_pool(name="pb", bufs=14))
            srowpool = ctx.enter_context(tc.tile_pool(name="srow", bufs=2))
            recpool = ctx.enter_context(tc.tile_pool(name="rec", bufs=2))
            rtpool = ctx.enter_context(tc.tile_pool(name="rt", bufs=4))
            rb16pool = ctx.enter_context(tc.tile_pool(name="rb16", bufs=3))
            pspool = ctx.enter_context(tc.tile_pool(name="ps", bufs=3, space="PSUM"))
            cxpool = ctx.enter_context(tc.tile_pool(name="cx", bufs=2, space="PSUM"))

            # ---- persistent small tiles ----
            idn_sb = persist.tile([128, 128], F32, tag="idn")
            nc.sync.dma_start(idn_sb, idn_d[:, :])
            bq2_sb = persist.tile([128, KT], F32, tag="bq2")
            nc.sync.dma_start(bq2_sb, bq2_d[:, :])
            bk2_sb = persist.tile([128, KT], F32, tag="bk2")
            nc.sync.dma_start(bk2_sb, bk2_d[:, :])
            bo2_sb = persist.tile([1, D], BF16, tag="bo2")
            nc.gpsimd.dma_start(out=bo2_sb, in_=bo2_d[:, :])  # f32 -> bf16 cast
            gamma_b = persist.tile([128, D], F32, tag="gamma_b")
            nc.sync.dma_start(gamma_b, gamma_d[0:1, :].to_broadcast([128, D]))
            beta_b = persist.tile([128, D], F32, tag="beta_b")
            nc.sync.dma_start(beta_b, beta_d[0:1, :].to_broadcast([128, D]))
            ones1 = persist.tile([1, 128], BF16, tag="ones1")
            nc.gpsimd.memset(ones1, 1.0)
            eps_sb = persist.tile([128, 1], F32, tag="eps")
            nc.vector.memset(eps_sb, EPS)

            # ---- persistent big tiles ----
            xT = [persist.tile([128, S], BF16, tag=f"xT{j}", name=f"xT{j}")
                  for j in range(KT)]
            for j in range(KT):
                nc.sync.dma_start(xT[j], xT_d[j * 128:(j + 1) * 128, :])
            # v with a ones column per head: [128, 16*(64+1)]
            vE = [persist.tile([128, H * 65], BF16, tag=f"vE{t}", name=f"vE{t}")
                  for t in range(KT)]
            for t in range(KT):
                nc.gpsimd.memset(
                    vE[t].rearrange("p (h x) -> p h x", x=65)[:, :, 64:65], 1.0
                )
            ctxT = [persist.tile([128, S], BF16, tag=f"ctxT{t}", name=f"ctxT{t}")
                    for t in range(PAIRS)]

            # ---- weights (bf16, resident; wo reuses wv slots) ----
            w_v = [wpool.tile([128, D], BF16, tag="w", name=f"wv{i}") for i in range(KT)]
            w_q = [wpool.tile([128, D], BF16, tag="w", name=f"wq{i}") for i in range(KT)]
            w_k = [wpool.tile([128, D], BF16, tag="w", name=f"wk{i}") for i in range(KT)]
            for kt in range(KT):
                nc.sync.dma_start(w_v[kt], wv_d[kt * 128:(kt + 1) * 128, :])
            for kt in range(KT):
                nc.sync.dma_start(w_q[kt], wq_d[kt * 128:(kt + 1) * 128, :])
                nc.sync.dma_start(w_k[kt], wk_d[kt * 128:(kt + 1) * 128, :])

            # ================= v projection (natural [s, d]) =================
            for t in range(ST):
                for dh in range(2):
                    ps = pspool.tile([128, 512], F32, tag="sc", name=f"vps{t}_{dh}")
                    for kt in range(KT):
                        nc.tensor.matmul(
                            ps,
                            lhsT=xT[kt][:, t * 128:(t + 1) * 128],
                            rhs=w_v[kt][:, dh * 512:(dh + 1) * 512],
                            start=(kt == 0),
                            stop=(kt == KT - 1),
                        )
                    nc.vector.tensor_copy(
                        vE[t][:, dh * 520:(dh + 1) * 520]
                        .rearrange("p (h x) -> p h x", x=65)[:, :, 0:64],
                        ps.rearrange("p (h x) -> p h x", x=64),
                    )

            w_o = [wpool.tile([128, D], BF16, tag="w", name=f"wo{i}") for i in range(KT)]
            for kt in range(KT):
                nc.sync.dma_start(w_o[kt], wo_d[kt * 128:(kt + 1) * 128, :])

            # ======== per-pair: q/k projection then attention (2 heads) ======
            for pair in range(PAIRS):
                heads = (2 * pair, 2 * pair + 1)
                qT2, kT2 = {}, {}
                for h in heads:
                    qT2[h] = qpool.tile([65, S], BF16, tag="qTe", name=f"qTe{h}")
                    nc.gpsimd.memset(qT2[h][64:65, :], 1.0)
                    kT2[h] = kpool.tile([65, S], BF16, tag="kTe", name=f"kTe{h}")
                    nc.gpsimd.dma_start(out=kT2[h][64:65, :], in_=bias8_d[0:1, :])

                for wi, (w_tiles, b_sb, dest) in enumerate(
                        ((w_q, bq2_sb, qT2), (w_k, bk2_sb, kT2))):
                    for sh in range(2):
                        ps = pspool.tile([128, 512], F32, tag="sc",
                                         name=f"qkps{pair}_{sh}")
                        for kt in range(KT):
                            nc.tensor.matmul(
                                ps,
                                lhsT=w_tiles[kt][:, pair * 128:(pair + 1) * 128],
                                rhs=xT[kt][:, sh * 512:(sh + 1) * 512],
                                start=(kt == 0),
                                stop=(kt == KT - 1),
                            )
                        for half in range(2):
                            h = 2 * pair + half
                            dst = dest[h][0:64, sh * 512:(sh + 1) * 512]
                            srcp = ps[half * 64:(half + 1) * 64, :]
                            if skip_bias:
                                if wi == 0:
                                    nc.scalar.copy(dst, srcp)
                                else:
                                    nc.vector.tensor_copy(dst, srcp)
                            elif wi == 0:
                                nc.scalar.activation(
                                    dst, srcp,
                                    mybir.ActivationFunctionType.Identity,
                                    bias=b_sb[half * 64:(half + 1) * 64,
                                              pair:pair + 1],
                                )
                            else:
                                nc.vector.tensor_scalar(
                                    dst, srcp,
                                    b_sb[half * 64:(half + 1) * 64, pair:pair + 1],
                                    None, op0=AluAdd,
                                )

                for h in heads:
                    qT, kT_ = qT2[h], kT2[h]
                    # --- scores^T [k, q] -> single exp -> pT (unnormalized) ---
                    pb = []
                    for kt in range(ST):
                        ps = pspool.tile([128, S], F32, tag="sc", name=f"sB{h}_{kt}")
                        for qh in range(2):
                            nc.tensor.matmul(
                                ps[:, qh * 512:(qh + 1) * 512],
                                lhsT=kT_[:, kt * 128:(kt + 1) * 128],
                                rhs=qT[:, qh * 512:(qh + 1) * 512],
                                start=True,
                                stop=True,
                            )
                        pt = ptpool.tile([128, S], BF16, tag="pt", name=f"pt{h}_{kt}")
                        nc.scalar.activation(pt, ps, Exp, scale=0.125)
                        pb.append(pt)
                    # --- ctx^T (+ sums in row 64 via the v ones column) ---
                    ctxE = []
                    for qh in range(2):
                        pc = cxpool.tile([65, 512], F32, tag="cx",
                                         name=f"ctx{h}_{qh}")
                        for kt in range(ST):
                            nc.tensor.matmul(
                                pc,
                                lhsT=vE[kt][:, h * 65:(h + 1) * 65],
                                rhs=pb[kt][:, qh * 512:(qh + 1) * 512],
                                start=(kt == 0),
                                stop=(kt == ST - 1),
                            )
                        ctxE.append(pc)
                    # copy ctx out unnormalized; frees the psum slots fast
                    half = h % 2
                    for qh in range(2):
                        nc.vector.tensor_copy(
                            ctxT[h // 2][half * 64:(half + 1) * 64,
                                         qh * 512:(qh + 1) * 512],
                            ctxE[qh][0:64, :],
                        )
                    # --- softmax denominators -> reciprocal, broadcast [q] ---
                    srow = srowpool.tile([1, S], F32, tag="srow", name=f"srow{h}")
                    for qh in range(2):
                        nc.vector.tensor_copy(
                            srow[0:1, qh * 512:(qh + 1) * 512], ctxE[qh][64:65, :]
                        )
                    tp = cxpool.tile([128, ST], F32, tag="cx", name=f"tps{h}")
                    for j in range(ST):
                        nc.tensor.transpose(
                            tp[:, j:j + 1],
                            srow[0:1, j * 128:(j + 1) * 128],
                            idn_sb[0:1, 0:1],
                        )
                    recs = recpool.tile([128, ST], F32, tag="recs", name=f"recs{h}")
                    nc.vector.reciprocal(recs, tp)
                    rtr = cxpool.tile([8, 128], F32, tag="cx", name=f"rtr{h}")
                    nc.tensor.transpose(rtr, recs, idn_sb)
                    rT16 = rtpool.tile([8, 128], BF16, tag="rT16", name=f"rT16{h}")
                    nc.vector.tensor_copy(rT16, rtr)
                    nc.sync.dma_start(
                        r16_d[h:h + 1, :].rearrange("a (b c) -> (a b) c", c=128), rT16
                    )
                    rb16 = rb16pool.tile([128, S], BF16, tag="rb16", name=f"rb16{h}")
                    nc.sync.dma_start(rb16, r16_d[h:h + 1, :].to_broadcast([128, S]))
                    # --- ctx normalize in place (sbuf, bf16) ---
                    for qh in range(2):
                        sl = ctxT[h // 2][half * 64:(half + 1) * 64,
                                          qh * 512:(qh + 1) * 512]
                        nc.vector.tensor_mul(
                            sl, sl,
                            rb16[half * 64:(half + 1) * 64,
                                 qh * 512:(qh + 1) * 512],
                        )
                    # --- attn^T normalize + store ---
                    for kt in range(ST):
                        nc.vector.tensor_mul(pb[kt], pb[kt], rb16)
                        nc.sync.dma_start(
                            attn_d[h, kt * 128:(kt + 1) * 128, :], pb[kt]
                        )

            # ============== out-proj + residual + layernorm ===================
            with tc.tile_pool(name="p3x", bufs=2) as x2pool, \
                 tc.tile_pool(name="p3z", bufs=2) as zpool, \
                 tc.tile_pool(name="p3st", bufs=4) as statpool:
                for t in range(ST):
                    x2 = x2pool.tile([128, D], F32, tag="x2", name=f"x2{t}")
                    nc.sync.dma_start(x2, x_d[t * 128:(t + 1) * 128, :])
                    z = zpool.tile([128, D], F32, tag="z", name=f"z{t}")
                    for dh in range(2):
                        ps = pspool.tile([128, 512], F32, tag="sc", name=f"y{t}_{dh}")
                        for ct in range(PAIRS):
                            nc.tensor.matmul(
                                ps,
                                lhsT=ctxT[ct][:, t * 128:(t + 1) * 128],
                                rhs=w_o[ct][:, dh * 512:(dh + 1) * 512],
                                start=(ct == 0),
                                stop=(skip_bo2 and ct == PAIRS - 1),
                            )
                        if not skip_bo2:
                            nc.tensor.matmul(
                                ps,
                                lhsT=ones1,
                                rhs=bo2_sb[0:1, dh * 512:(dh + 1) * 512],
                                start=False,
                                stop=True,
                            )
                        nc.vector.tensor_add(
                            z[:, dh * 512:(dh + 1) * 512],
                            x2[:, dh * 512:(dh + 1) * 512],
                            ps,
                        )
                    stats = statpool.tile([128, 2, 6], F32, tag="stats", name=f"st{t}")
                    for sg in range(2):
                        nc.vector.bn_stats(
                            stats[:, sg, :], z[:, sg * 512:(sg + 1) * 512]
                        )
                    mv = statpool.tile([128, 2], F32, tag="mv", name=f"mv{t}")
                    nc.vector.bn_aggr(mv, stats)
                    sd = statpool.tile([128, 1], F32, tag="sd", name=f"sd{t}")
                    nc.scalar.activation(sd, mv[:, 1:2], SqrtF, bias=eps_sb)
                    rs = statpool.tile([128, 1], F32, tag="rs", name=f"rs{t}")
                    nc.vector.reciprocal(rs, sd)
                    nc.vector.tensor_scalar(
                        z, z, mv[:, 0:1], rs,
                        op0=mybir.AluOpType.subtract,
                        op1=mybir.AluOpType.mult,
                    )
                    if not skip_gamma:
                        nc.vector.tensor_mul(z, z, gamma_b)
                        nc.vector.tensor_add(z, z, beta_b)
                    nc.sync.dma_start(out_d[t * 128:(t + 1) * 128, :], z)

    nc.compile()
    return nc


def _get_nc(skip_bias=False, skip_gamma=False, skip_bo2=False):
    key = ("nc", skip_bias, skip_gamma, skip_bo2)
    if key not in _CACHE:
        _CACHE[key] = _build(skip_bias, skip_gamma, skip_bo2)
    return _CACHE[key]


def _install_ntff_hook():
    """Provide the antenv.axon_hooks shim the boot image lacks, so
    run_bass_kernel_spmd(trace=True) can capture NTFF profiles."""
    try:
        import types

        try:
            from antenv.axon_hooks import get_axon_ntff_profile_hook  # noqa: F401
        except ImportError:
            import antenv

            mod = types.ModuleType("antenv.axon_hooks")
            _hook = [None]
            mod.set_axon_ntff_profile_hook = lambda h: _hook.__setitem__(0, h)
            mod.get_axon_ntff_profile_hook = lambda: _hook[0]
            sys.modules["antenv.axon_hooks"] = mod
            antenv.axon_hooks = mod
        from antenv import axon_hooks

        if axon_hooks.get_axon_ntff_profile_hook() is None:
            from trn_agent_boot.trn_boot import _ntff_profile_via_ctypes

            hook = _ntff_profile_via_ctypes("/opt/axon/libaxon_pjrt.so")
            if hook is None:
                return False
            axon_hooks.set_axon_ntff_profile_hook(hook)
        import concourse.bass_utils as bu

        bu.upload_artifacts = lambda tmpdir: f"local:{tmpdir}"
        return True
    except Exception:
        import traceback

        traceback.print_exc()
        return False


def _to_bf16(a):
    import ml_dtypes

    return np.asarray(a, np.float32).astype(ml_dtypes.bfloat16)


def kernel(x, mask, wq, bq, wk, bk, wv, bv, wo, bo, gamma, beta):
    x = np.asarray(x, np.float32)
    mask = np.asarray(mask)
    wq, wk, wv, wo = (np.asarray(w, np.float32) for w in (wq, wk, wv, wo))
    bq, bk, bv, bo = (np.asarray(b, np.float32) for b in (bq, bk, bv, bo))
    gamma = np.asarray(gamma, np.float32)
    beta = np.asarray(beta, np.float32)

    skip_bias = bool(np.all(bq == 0.0) and np.all(bk == 0.0))
    skip_gamma = bool(np.all(gamma == 1.0) and np.all(beta == 0.0))
    bo2_chk = bo.astype(np.float64) + bv.astype(np.float64) @ wo.astype(np.float64)
    skip_bo2 = bool(np.all(bo2_chk == 0.0))
    nc = _get_nc(skip_bias, skip_gamma, skip_bo2)

    bo2 = (bo.astype(np.float64) + bv.astype(np.float64) @ wo.astype(np.float64))
    bo2 = bo2.astype(np.float32).reshape(1, D)
    shared = {
        "wq": _to_bf16(wq), "wk": _to_bf16(wk),
        "wv": _to_bf16(wv), "wo": _to_bf16(wo),
        "bq2": np.ascontiguousarray(bq.reshape(KT, 128).T),
        "bk2": np.ascontiguousarray(bk.reshape(KT, 128).T),
        "bo2": bo2,
        "gamma_r": gamma.reshape(1, D),
        "beta_r": beta.reshape(1, D),
    }
    in_maps = []
    for c in range(B):
        m = dict(shared)
        m["x"] = np.ascontiguousarray(x[c])
        m["xT"] = np.ascontiguousarray(_to_bf16(x[c].T))
        m["bias8"] = (NINF_BIAS * mask[c].astype(np.float32)).reshape(1, S)
        in_maps.append(m)

    trace = bool(int(os.environ.get("KERNEL_TRACE", "0")))
    if trace:
        trace = _install_ntff_hook()
    try:
        res = run_bass_kernel_spmd(
            nc, in_maps, core_ids=list(range(B)), trace=trace,
        )
    except Exception:
        if not trace:
            raise
        import traceback

        traceback.print_exc()
        res = run_bass_kernel_spmd(
            nc, in_maps, core_ids=list(range(B)), trace=False,
        )
    _CACHE["last_result"] = res

    out = np.stack([np.asarray(res.results[c]["out"], np.float32) for c in range(B)])
    # attn comes back per-head transposed [H, k, q]; swap back to [H, q, k]
    attn = np.empty((B, H, S, S), np.float32)
    for c in range(B):
        attn[c] = np.asarray(res.results[c]["attn"]).astype(np.float32).swapaxes(1, 2)
    return out, attn


# revision 16
# speedup vs baseline: 1.4415x; 1.1316x over previous
"""Trainium2 Bass kernel for fused multi-head attention block.

Per batch element b (one NeuronCore per element, 8 cores, pure data
parallelism, no collectives):

  q = x @ wq + bq ; k = x @ wk + bk ; v = x @ wv + bv      (16 heads x 64)
  scores = q k^T / sqrt(64) - 10000 * mask[k]
  attn   = softmax(scores)          (no max-subtraction; masked cols -> exact 0)
  ctx    = attn @ v
  out    = LayerNorm(x + ctx @ wo + bo) * gamma + beta

Everything is computed in the transposed orientation [k, q] so a single
exp pass feeds both the ctx matmul and the attention-probability output:
  - scores^T via a K=65 matmul (the mask bias rides as a 65th contraction
    row: kT row 64 = -80000*mask, qT row 64 = ones),
  - softmax denominators come free as row 64 of the ctx psum (a ones
    column appended to each v tile),
  - the reciprocal is broadcast across partitions via a tiny DRAM bounce,
  - attn is written to DRAM transposed; the host unshard step transposes
    it back (layout only, no FLOPs).
Host pre-transposes x and pre-casts weights to bf16 (marshalling only).
"""

import os
import sys

import numpy as np

for _p in ("/opt/trn_rl_repo",):
    if _p not in sys.path:
        sys.path.insert(0, _p)

import concourse.bass as bass  # noqa: E402
import concourse.tile as tile  # noqa: E402
from concourse import mybir  # noqa: E402
from concourse.bacc import Bacc  # noqa: E402
from concourse.bass_utils import run_bass_kernel_spmd  # noqa: E402

B, S, H, HS = 8, 1024, 16, 64
D = H * HS
KT = D // 128          # contraction tiles of 128
ST = S // 128          # sequence tiles of 128
PAIRS = H // 2
EPS = 1e-6
NINF_BIAS = -80000.0   # exp(0.125*(qk + bias_row)) == exp(qk/8 - 10000*mask)

F32 = mybir.dt.float32
BF16 = mybir.dt.bfloat16

_CACHE = {}


def _build(skip_bias=False, skip_gamma=False, skip_bo2=False):
    nc = Bacc("TRN2", target_bir_lowering=False, debug=False, enable_asserts=False)

    x_d = nc.dram_tensor("x", [S, D], F32, kind="ExternalInput")
    xT_d = nc.dram_tensor("xT", [D, S], BF16, kind="ExternalInput")
    bias8_d = nc.dram_tensor("bias8", [1, S], F32, kind="ExternalInput")
    wq_d = nc.dram_tensor("wq", [D, D], BF16, kind="ExternalInput")
    wk_d = nc.dram_tensor("wk", [D, D], BF16, kind="ExternalInput")
    wv_d = nc.dram_tensor("wv", [D, D], BF16, kind="ExternalInput")
    wo_d = nc.dram_tensor("wo", [D, D], BF16, kind="ExternalInput")
    bq2_d = nc.dram_tensor("bq2", [128, KT], F32, kind="ExternalInput")
    bk2_d = nc.dram_tensor("bk2", [128, KT], F32, kind="ExternalInput")
    bo2_d = nc.dram_tensor("bo2", [1, D], F32, kind="ExternalInput")
    gamma_d = nc.dram_tensor("gamma_r", [1, D], F32, kind="ExternalInput")
    beta_d = nc.dram_tensor("beta_r", [1, D], F32, kind="ExternalInput")

    out_d = nc.dram_tensor("out", [S, D], F32, kind="ExternalOutput")
    # attn stored TRANSPOSED per head: attn_d[h, k, q]; host swaps back.
    attn_d = nc.dram_tensor("attn", [H, S, S], BF16, kind="ExternalOutput")
    r16_d = nc.dram_tensor("r16_scratch", [H, S], BF16)

    idn_d = nc.inline_tensor(np.eye(128, dtype=np.float32), name="idn")

    Exp = mybir.ActivationFunctionType.Exp
    SqrtF = mybir.ActivationFunctionType.Sqrt
    AluAdd = mybir.AluOpType.add

    with tile.TileContext(nc) as tc:
        from contextlib import ExitStack

        with ExitStack() as ctx:
            persist = ctx.enter_context(tc.tile_pool(name="persist", bufs=1))
            wpool = ctx.enter_context(tc.tile_pool(name="w", bufs=24))
            qpool = ctx.enter_context(tc.tile_pool(name="qp", bufs=6))
            kpool = ctx.enter_context(tc.tile_pool(name="kp", bufs=6))
            ptpool = ctx.enter_context(tc.tile_pool(name="pb", bufs=16))
            srowpool = ctx.enter_context(tc.tile_pool(name="srow", bufs=2))
            recpool = ctx.enter_context(tc.tile_pool(name="rec", bufs=2))
            rtpool = ctx.enter_context(tc.tile_pool(name="rt", bufs=4))
            rb16pool = ctx.enter_context(tc.tile_pool(name="rb16", bufs=4))
            pspool = ctx.enter_context(tc.tile_pool(name="ps", bufs=3, space="PSUM"))
            cxpool = ctx.enter_context(tc.tile_pool(name="cx", bufs=2, space="PSUM"))

            # ---- persistent big tiles ----
            xT = [persist.tile([128, S], BF16, tag=f"xT{j}", name=f"xT{j}")
                  for j in range(KT)]
            for j in range(KT):
                nc.sync.dma_start(xT[j], xT_d[j * 128:(j + 1) * 128, :])
            # v with a ones column per head: [128, 16*(64+1)]
            vE = [persist.tile([128, H * 65], BF16, tag=f"vE{t}", name=f"vE{t}")
                  for t in range(KT)]
            for t in range(KT):
                nc.gpsimd.memset(
                    vE[t].rearrange("p (h x) -> p h x", x=65)[:, :, 64:65], 1.0
                )
            ctxT = [persist.tile([128, S], BF16, tag=f"ctxT{t}", name=f"ctxT{t}")
                    for t in range(PAIRS)]

            # ---- weights (bf16, resident; wo reuses wv slots) ----
            w_v = [wpool.tile([128, D], BF16, tag="w", name=f"wv{i}") for i in range(KT)]
            w_q = [wpool.tile([128, D], BF16, tag="w", name=f"wq{i}") for i in range(KT)]
            w_k = [wpool.tile([128, D], BF16, tag="w", name=f"wk{i}") for i in range(KT)]
            for kt in range(KT):
                nc.sync.dma_start(w_v[kt], wv_d[kt * 128:(kt + 1) * 128, :])
            for kt in range(KT):
                nc.sync.dma_start(w_q[kt], wq_d[kt * 128:(kt + 1) * 128, :])
                nc.sync.dma_start(w_k[kt], wk_d[kt * 128:(kt + 1) * 128, :])

            # ---- persistent small tiles ----
            idn_sb = persist.tile([128, 128], F32, tag="idn")
            nc.sync.dma_start(idn_sb, idn_d[:, :])
            bq2_sb = persist.tile([128, KT], F32, tag="bq2")
            nc.sync.dma_start(bq2_sb, bq2_d[:, :])
            bk2_sb = persist.tile([128, KT], F32, tag="bk2")
            nc.sync.dma_start(bk2_sb, bk2_d[:, :])
            bo2_sb = persist.tile([1, D], BF16, tag="bo2")
            nc.gpsimd.dma_start(out=bo2_sb, in_=bo2_d[:, :])  # f32 -> bf16 cast
            gamma_b = persist.tile([128, D], F32, tag="gamma_b")
            nc.sync.dma_start(gamma_b, gamma_d[0:1, :].to_broadcast([128, D]))
            beta_b = persist.tile([128, D], F32, tag="beta_b")
            nc.sync.dma_start(beta_b, beta_d[0:1, :].to_broadcast([128, D]))
            ones1 = persist.tile([1, 128], BF16, tag="ones1")
            nc.gpsimd.memset(ones1, 1.0)
            eps_sb = persist.tile([128, 1], F32, tag="eps")
            nc.vector.memset(eps_sb, EPS)


            # ================= v projection (natural [s, d]) =================
            for t in range(ST):
                for dh in range(2):
                    ps = pspool.tile([128, 512], F32, tag="sc", name=f"vps{t}_{dh}")
                    for kt in range(KT):
                        nc.tensor.matmul(
                            ps,
                            lhsT=xT[kt][:, t * 128:(t + 1) * 128],
                            rhs=w_v[kt][:, dh * 512:(dh + 1) * 512],
                            start=(kt == 0),
                            stop=(kt == KT - 1),
                        )
                    nc.vector.tensor_copy(
                        vE[t][:, dh * 520:(dh + 1) * 520]
                        .rearrange("p (h x) -> p h x", x=65)[:, :, 0:64],
                        ps.rearrange("p (h x) -> p h x", x=64),
                    )

            w_o = [wpool.tile([128, D], BF16, tag="w", name=f"wo{i}") for i in range(KT)]
            for kt in range(KT):
                nc.sync.dma_start(w_o[kt], wo_d[kt * 128:(kt + 1) * 128, :])

            # ======== per-pair: q/k projection then attention (2 heads) ======
            for pair in range(PAIRS):
                heads = (2 * pair, 2 * pair + 1)
                qT2, kT2 = {}, {}
                for h in heads:
                    qT2[h] = qpool.tile([65, S], BF16, tag="qTe", name=f"qTe{h}")
                    nc.gpsimd.memset(qT2[h][64:65, :], 1.0)
                    kT2[h] = kpool.tile([65, S], BF16, tag="kTe", name=f"kTe{h}")
                    nc.gpsimd.dma_start(out=kT2[h][64:65, :], in_=bias8_d[0:1, :])

                for wi, (w_tiles, b_sb, dest) in enumerate(
                        ((w_q, bq2_sb, qT2), (w_k, bk2_sb, kT2))):
                    for sh in range(2):
                        ps = pspool.tile([128, 512], F32, tag="sc",
                                         name=f"qkps{pair}_{sh}")
                        for kt in range(KT):
                            nc.tensor.matmul(
                                ps,
                                lhsT=w_tiles[kt][:, pair * 128:(pair + 1) * 128],
                                rhs=xT[kt][:, sh * 512:(sh + 1) * 512],
                                start=(kt == 0),
                                stop=(kt == KT - 1),
                            )
                        for half in range(2):
                            h = 2 * pair + half
                            dst = dest[h][0:64, sh * 512:(sh + 1) * 512]
                            srcp = ps[half * 64:(half + 1) * 64, :]
                            if skip_bias:
                                if wi == 0:
                                    nc.scalar.copy(dst, srcp)
                                else:
                                    nc.vector.tensor_copy(dst, srcp)
                            elif wi == 0:
                                nc.scalar.activation(
                                    dst, srcp,
                                    mybir.ActivationFunctionType.Identity,
                                    bias=b_sb[half * 64:(half + 1) * 64,
                                              pair:pair + 1],
                                )
                            else:
                                nc.vector.tensor_scalar(
                                    dst, srcp,
                                    b_sb[half * 64:(half + 1) * 64, pair:pair + 1],
                                    None, op0=AluAdd,
                                )

                for h in heads:
                    qT, kT_ = qT2[h], kT2[h]
                    # --- scores^T [k, q] -> single exp -> pT (unnormalized) ---
                    pb = []
                    for kt in range(ST):
                        ps = pspool.tile([128, S], F32, tag="sc", name=f"sB{h}_{kt}")
                        for qh in range(2):
                            nc.tensor.matmul(
                                ps[:, qh * 512:(qh + 1) * 512],
                                lhsT=kT_[:, kt * 128:(kt + 1) * 128],
                                rhs=qT[:, qh * 512:(qh + 1) * 512],
                                start=True,
                                stop=True,
                            )
                        pt = ptpool.tile([128, S], BF16, tag="pt", name=f"pt{h}_{kt}")
                        nc.scalar.activation(pt, ps, Exp, scale=0.125)
                        pb.append(pt)
                    # --- ctx^T (+ sums in row 64 via the v ones column) ---
                    ctxE = []
                    for qh in range(2):
                        pc = cxpool.tile([65, 512], F32, tag="cx",
                                         name=f"ctx{h}_{qh}")
                        for kt in range(ST):
                            nc.tensor.matmul(
                                pc,
                                lhsT=vE[kt][:, h * 65:(h + 1) * 65],
                                rhs=pb[kt][:, qh * 512:(qh + 1) * 512],
                                start=(kt == 0),
                                stop=(kt == ST - 1),
                            )
                        ctxE.append(pc)
                    # copy ctx out unnormalized; frees the psum slots fast
                    half = h % 2
                    for qh in range(2):
                        nc.vector.tensor_copy(
                            ctxT[h // 2][half * 64:(half + 1) * 64,
                                         qh * 512:(qh + 1) * 512],
                            ctxE[qh][0:64, :],
                        )
                    # --- softmax denominators -> reciprocal, broadcast [q] ---
                    srow = srowpool.tile([1, S], F32, tag="srow", name=f"srow{h}")
                    for qh in range(2):
                        nc.vector.tensor_copy(
                            srow[0:1, qh * 512:(qh + 1) * 512], ctxE[qh][64:65, :]
                        )
                    tp = cxpool.tile([128, ST], F32, tag="cx", name=f"tps{h}")
                    for j in range(ST):
                        nc.tensor.transpose(
                            tp[:, j:j + 1],
                            srow[0:1, j * 128:(j + 1) * 128],
                            idn_sb[0:1, 0:1],
                        )
                    recs = recpool.tile([128, ST], F32, tag="recs", name=f"recs{h}")
                    nc.vector.reciprocal(recs, tp)
                    rtr = cxpool.tile([8, 128], F32, tag="cx", name=f"rtr{h}")
                    nc.tensor.transpose(rtr, recs, idn_sb)
                    rT16 = rtpool.tile([8, 128], BF16, tag="rT16", name=f"rT16{h}")
                    nc.vector.tensor_copy(rT16, rtr)
                    nc.sync.dma_start(
                        r16_d[h:h + 1, :].rearrange("a (b c) -> (a b) c", c=128), rT16
                    )
                    rb16 = rb16pool.tile([128, S], BF16, tag="rb16", name=f"rb16{h}")
                    nc.sync.dma_start(rb16, r16_d[h:h + 1, :].to_broadcast([128, S]))
                    # --- ctx normalize in place (sbuf, bf16) ---
                    for qh in range(2):
                        sl = ctxT[h // 2][half * 64:(half + 1) * 64,
                                          qh * 512:(qh + 1) * 512]
                        nc.vector.tensor_mul(
                            sl, sl,
                            rb16[half * 64:(half + 1) * 64,
                                 qh * 512:(qh + 1) * 512],
                        )
                    # --- attn^T normalize + store ---
                    for kt in range(ST):
                        nc.vector.tensor_mul(pb[kt], pb[kt], rb16)
                        nc.sync.dma_start(
                            attn_d[h, kt * 128:(kt + 1) * 128, :], pb[kt]
                        )

            # ============== out-proj + residual + layernorm ===================
            with tc.tile_pool(name="p3x", bufs=2) as x2pool, \
                 tc.tile_pool(name="p3z", bufs=2) as zpool, \
                 tc.tile_pool(name="p3st", bufs=4) as statpool:
                for t in range(ST):
                    x2 = x2pool.tile([128, D], F32, tag="x2", name=f"x2{t}")
                    nc.sync.dma_start(x2, x_d[t * 128:(t + 1) * 128, :])
                    z = zpool.tile([128, D], F32, tag="z", name=f"z{t}")
                    for dh in range(2):
                        ps = pspool.tile([128, 512], F32, tag="sc", name=f"y{t}_{dh}")
                        for ct in range(PAIRS):
                            nc.tensor.matmul(
                                ps,
                                lhsT=ctxT[ct][:, t * 128:(t + 1) * 128],
                                rhs=w_o[ct][:, dh * 512:(dh + 1) * 512],
                                start=(ct == 0),
                                stop=(skip_bo2 and ct == PAIRS - 1),
                            )
                        if not skip_bo2:
                            nc.tensor.matmul(
                                ps,
                                lhsT=ones1,
                                rhs=bo2_sb[0:1, dh * 512:(dh + 1) * 512],
                                start=False,
                                stop=True,
                            )
                        nc.vector.tensor_add(
                            z[:, dh * 512:(dh + 1) * 512],
                            x2[:, dh * 512:(dh + 1) * 512],
                            ps,
                        )
                    stats = statpool.tile([128, 2, 6], F32, tag="stats", name=f"st{t}")
                    for sg in range(2):
                        nc.vector.bn_stats(
                            stats[:, sg, :], z[:, sg * 512:(sg + 1) * 512]
                        )
                    mv = statpool.tile([128, 2], F32, tag="mv", name=f"mv{t}")
                    nc.vector.bn_aggr(mv, stats)
                    sd = statpool.tile([128, 1], F32, tag="sd", name=f"sd{t}")
                    nc.scalar.activation(sd, mv[:, 1:2], SqrtF, bias=eps_sb)
                    rs = statpool.tile([128, 1], F32, tag="rs", name=f"rs{t}")
                    nc.vector.reciprocal(rs, sd)
                    nc.vector.tensor_scalar(
                        z, z, mv[:, 0:1], rs,
                        op0=mybir.AluOpType.subtract,
                        op1=mybir.AluOpType.mult,
                    )
                    if not skip_gamma:
                        nc.vector.tensor_mul(z, z, gamma_b)
                        nc.vector.tensor_add(z, z, beta_b)
                    nc.sync.dma_start(out_d[t * 128:(t + 1) * 128, :], z)

    nc.compile()
    return nc


def _get_nc(skip_bias=False, skip_gamma=False, skip_bo2=False):
    key = ("nc", skip_bias, skip_gamma, skip_bo2)
    if key not in _CACHE:
        _CACHE[key] = _build(skip_bias, skip_gamma, skip_bo2)
    return _CACHE[key]


def _install_ntff_hook():
    """Provide the antenv.axon_hooks shim the boot image lacks, so
    run_bass_kernel_spmd(trace=True) can capture NTFF profiles."""
    try:
        import types

        try:
            from antenv.axon_hooks import get_axon_ntff_profile_hook  # noqa: F401
        except ImportError:
            import antenv

            mod = types.ModuleType("antenv.axon_hooks")
            _hook = [None]
            mod.set_axon_ntff_profile_hook = lambda h: _hook.__setitem__(0, h)
            mod.get_axon_ntff_profile_hook = lambda: _hook[0]
            sys.modules["antenv.axon_hooks"] = mod
            antenv.axon_hooks = mod
        from antenv import axon_hooks

        if axon_hooks.get_axon_ntff_profile_hook() is None:
            from trn_agent_boot.trn_boot import _ntff_profile_via_ctypes

            hook = _ntff_profile_via_ctypes("/opt/axon/libaxon_pjrt.so")
            if hook is None:
                return False
            axon_hooks.set_axon_ntff_profile_hook(hook)
        import concourse.bass_utils as bu

        bu.upload_artifacts = lambda tmpdir: f"local:{tmpdir}"
        return True
    except Exception:
        import traceback

        traceback.print_exc()
        return False


def _to_bf16(a):
    import ml_dtypes

    return np.asarray(a, np.float32).astype(ml_dtypes.bfloat16)


def kernel(x, mask, wq, bq, wk, bk, wv, bv, wo, bo, gamma, beta):
    x = np.asarray(x, np.float32)
    mask = np.asarray(mask)
    wq, wk, wv, wo = (np.asarray(w, np.float32) for w in (wq, wk, wv, wo))
    bq, bk, bv, bo = (np.asarray(b, np.float32) for b in (bq, bk, bv, bo))
    gamma = np.asarray(gamma, np.float32)
    beta = np.asarray(beta, np.float32)

    skip_bias = bool(np.all(bq == 0.0) and np.all(bk == 0.0))
    skip_gamma = bool(np.all(gamma == 1.0) and np.all(beta == 0.0))
    bo2_chk = bo.astype(np.float64) + bv.astype(np.float64) @ wo.astype(np.float64)
    skip_bo2 = bool(np.all(bo2_chk == 0.0))
    nc = _get_nc(skip_bias, skip_gamma, skip_bo2)

    bo2 = (bo.astype(np.float64) + bv.astype(np.float64) @ wo.astype(np.float64))
    bo2 = bo2.astype(np.float32).reshape(1, D)
    shared = {
        "wq": _to_bf16(wq), "wk": _to_bf16(wk),
        "wv": _to_bf16(wv), "wo": _to_bf16(wo),
        "bq2": np.ascontiguousarray(bq.reshape(KT, 128).T),
        "bk2": np.ascontiguousarray(bk.reshape(KT, 128).T),
        "bo2": bo2,
        "gamma_r": gamma.reshape(1, D),
        "beta_r": beta.reshape(1, D),
    }
    in_maps = []
    for c in range(B):
        m = dict(shared)
        m["x"] = np.ascontiguousarray(x[c])
        m["xT"] = np.ascontiguousarray(_to_bf16(x[c].T))
        m["bias8"] = (NINF_BIAS * mask[c].astype(np.float32)).reshape(1, S)
        in_maps.append(m)

    trace = bool(int(os.environ.get("KERNEL_TRACE", "0")))
    if trace:
        trace = _install_ntff_hook()
    try:
        res = run_bass_kernel_spmd(
            nc, in_maps, core_ids=list(range(B)), trace=trace,
        )
    except Exception:
        if not trace:
            raise
        import traceback

        traceback.print_exc()
        res = run_bass_kernel_spmd(
            nc, in_maps, core_ids=list(range(B)), trace=False,
        )
    _CACHE["last_result"] = res

    out = np.stack([np.asarray(res.results[c]["out"], np.float32) for c in range(B)])
    # attn comes back per-head transposed [H, k, q]; swap back to [H, q, k]
    attn = np.empty((B, H, S, S), np.float32)
    for c in range(B):
        attn[c] = np.asarray(res.results[c]["attn"]).astype(np.float32).swapaxes(1, 2)
    return out, attn


# revision 17
# speedup vs baseline: 1.4531x; 1.0081x over previous
"""Trainium2 Bass kernel for fused multi-head attention block.

Per batch element b (one NeuronCore per element, 8 cores, pure data
parallelism, no collectives):

  q = x @ wq + bq ; k = x @ wk + bk ; v = x @ wv + bv      (16 heads x 64)
  scores = q k^T / sqrt(64) - 10000 * mask[k]
  attn   = softmax(scores)          (no max-subtraction; masked cols -> exact 0)
  ctx    = attn @ v
  out    = LayerNorm(x + ctx @ wo + bo) * gamma + beta

Everything is computed in the transposed orientation [k, q] so a single
exp pass feeds both the ctx matmul and the attention-probability output:
  - scores^T via a K=65 matmul (the mask bias rides as a 65th contraction
    row: kT row 64 = -80000*mask, qT row 64 = ones),
  - softmax denominators come free as row 64 of the ctx psum (a ones
    column appended to each v tile),
  - the reciprocal is broadcast across partitions via a tiny DRAM bounce,
  - attn is written to DRAM transposed; the host unshard step transposes
    it back (layout only, no FLOPs).
Host pre-transposes x and pre-casts weights to bf16 (marshalling only).
"""

import os
import sys

import numpy as np

for _p in ("/opt/trn_rl_repo",):
    if _p not in sys.path:
        sys.path.insert(0, _p)

import concourse.bass as bass  # noqa: E402
import concourse.tile as tile  # noqa: E402
from concourse import mybir  # noqa: E402
from concourse.bacc import Bacc  # noqa: E402
from concourse.bass_utils import run_bass_kernel_spmd  # noqa: E402

B, S, H, HS = 8, 1024, 16, 64
D = H * HS
KT = D // 128          # contraction tiles of 128
ST = S // 128          # sequence tiles of 128
PAIRS = H // 2
EPS = 1e-6
NINF_BIAS = -80000.0   # exp(0.125*(qk + bias_row)) == exp(qk/8 - 10000*mask)

F32 = mybir.dt.float32
BF16 = mybir.dt.bfloat16

_CACHE = {}


def _build(skip_bias=False, skip_gamma=False, skip_bo2=False):
    nc = Bacc("TRN2", target_bir_lowering=False, debug=False, enable_asserts=False)

    x_d = nc.dram_tensor("x", [S, D], F32, kind="ExternalInput")
    xT_d = nc.dram_tensor("xT", [D, S], BF16, kind="ExternalInput")
    bias8_d = nc.dram_tensor("bias8", [1, S], F32, kind="ExternalInput")
    wq_d = nc.dram_tensor("wq", [D, D], BF16, kind="ExternalInput")
    wk_d = nc.dram_tensor("wk", [D, D], BF16, kind="ExternalInput")
    wv_d = nc.dram_tensor("wv", [D, D], BF16, kind="ExternalInput")
    wo_d = nc.dram_tensor("wo", [D, D], BF16, kind="ExternalInput")
    bq2_d = nc.dram_tensor("bq2", [128, KT], F32, kind="ExternalInput")
    bk2_d = nc.dram_tensor("bk2", [128, KT], F32, kind="ExternalInput")
    bo2_d = nc.dram_tensor("bo2", [1, D], F32, kind="ExternalInput")
    gamma_d = nc.dram_tensor("gamma_r", [1, D], F32, kind="ExternalInput")
    beta_d = nc.dram_tensor("beta_r", [1, D], F32, kind="ExternalInput")

    out_d = nc.dram_tensor("out", [S, D], F32, kind="ExternalOutput")
    # attn stored TRANSPOSED per head: attn_d[h, k, q]; host swaps back.
    attn_d = nc.dram_tensor("attn", [H, S, S], BF16, kind="ExternalOutput")
    r16_d = nc.dram_tensor("r16_scratch", [H, S], BF16)

    idn_d = nc.inline_tensor(np.eye(128, dtype=np.float32), name="idn")

    Exp = mybir.ActivationFunctionType.Exp
    SqrtF = mybir.ActivationFunctionType.Sqrt
    AluAdd = mybir.AluOpType.add

    with tile.TileContext(nc) as tc:
        from contextlib import ExitStack

        with ExitStack() as ctx:
            persist = ctx.enter_context(tc.tile_pool(name="persist", bufs=1))
            wpool = ctx.enter_context(tc.tile_pool(name="w", bufs=24))
            qpool = ctx.enter_context(tc.tile_pool(name="qp", bufs=7))
            kpool = ctx.enter_context(tc.tile_pool(name="kp", bufs=7))
            ptpool = ctx.enter_context(tc.tile_pool(name="pb", bufs=18))
            srowpool = ctx.enter_context(tc.tile_pool(name="srow", bufs=2))
            recpool = ctx.enter_context(tc.tile_pool(name="rec", bufs=2))
            rtpool = ctx.enter_context(tc.tile_pool(name="rt", bufs=6))
            rb16pool = ctx.enter_context(tc.tile_pool(name="rb16", bufs=5))
            pspool = ctx.enter_context(tc.tile_pool(name="ps", bufs=3, space="PSUM"))
            cxpool = ctx.enter_context(tc.tile_pool(name="cx", bufs=2, space="PSUM"))

            # ---- persistent big tiles ----
            xT = [persist.tile([128, S], BF16, tag=f"xT{j}", name=f"xT{j}")
                  for j in range(KT)]
            for j in range(KT):
                nc.sync.dma_start(xT[j], xT_d[j * 128:(j + 1) * 128, :])
            # v with a ones column per head: [128, 16*(64+1)]
            vE = [persist.tile([128, H * 65], BF16, tag=f"vE{t}", name=f"vE{t}")
                  for t in range(KT)]
            for t in range(KT):
                nc.gpsimd.memset(
                    vE[t].rearrange("p (h x) -> p h x", x=65)[:, :, 64:65], 1.0
                )
            ctxT = [persist.tile([128, S], BF16, tag=f"ctxT{t}", name=f"ctxT{t}")
                    for t in range(PAIRS)]

            # ---- weights (bf16, resident; wo reuses wv slots) ----
            w_v = [wpool.tile([128, D], BF16, tag="w", name=f"wv{i}") for i in range(KT)]
            w_q = [wpool.tile([128, D], BF16, tag="w", name=f"wq{i}") for i in range(KT)]
            w_k = [wpool.tile([128, D], BF16, tag="w", name=f"wk{i}") for i in range(KT)]
            for kt in range(KT):
                nc.sync.dma_start(w_v[kt], wv_d[kt * 128:(kt + 1) * 128, :])
            for kt in range(KT):
                nc.sync.dma_start(w_q[kt], wq_d[kt * 128:(kt + 1) * 128, :])
                nc.sync.dma_start(w_k[kt], wk_d[kt * 128:(kt + 1) * 128, :])

            # ---- persistent small tiles ----
            idn_sb = persist.tile([128, 128], F32, tag="idn")
            nc.sync.dma_start(idn_sb, idn_d[:, :])
            bq2_sb = persist.tile([128, KT], F32, tag="bq2")
            nc.sync.dma_start(bq2_sb, bq2_d[:, :])
            bk2_sb = persist.tile([128, KT], F32, tag="bk2")
            nc.sync.dma_start(bk2_sb, bk2_d[:, :])
            bo2_sb = persist.tile([1, D], BF16, tag="bo2")
            nc.gpsimd.dma_start(out=bo2_sb, in_=bo2_d[:, :])  # f32 -> bf16 cast
            gamma_b = persist.tile([128, D], F32, tag="gamma_b")
            nc.sync.dma_start(gamma_b, gamma_d[0:1, :].to_broadcast([128, D]))
            beta_b = persist.tile([128, D], F32, tag="beta_b")
            nc.sync.dma_start(beta_b, beta_d[0:1, :].to_broadcast([128, D]))
            ones1 = persist.tile([1, 128], BF16, tag="ones1")
            nc.gpsimd.memset(ones1, 1.0)
            eps_sb = persist.tile([128, 1], F32, tag="eps")
            nc.vector.memset(eps_sb, EPS)


            # ================= v projection (natural [s, d]) =================
            for t in range(ST):
                for dh in range(2):
                    ps = pspool.tile([128, 512], F32, tag="sc", name=f"vps{t}_{dh}")
                    for kt in range(KT):
                        nc.tensor.matmul(
                            ps,
                            lhsT=xT[kt][:, t * 128:(t + 1) * 128],
                            rhs=w_v[kt][:, dh * 512:(dh + 1) * 512],
                            start=(kt == 0),
                            stop=(kt == KT - 1),
                        )
                    nc.vector.tensor_copy(
                        vE[t][:, dh * 520:(dh + 1) * 520]
                        .rearrange("p (h x) -> p h x", x=65)[:, :, 0:64],
                        ps.rearrange("p (h x) -> p h x", x=64),
                    )

            w_o = [wpool.tile([128, D], BF16, tag="w", name=f"wo{i}") for i in range(KT)]
            for kt in range(KT):
                nc.sync.dma_start(w_o[kt], wo_d[kt * 128:(kt + 1) * 128, :])

            # ======== per-pair: q/k projection then attention (2 heads) ======
            for pair in range(PAIRS):
                heads = (2 * pair, 2 * pair + 1)
                qT2, kT2 = {}, {}
                for h in heads:
                    qT2[h] = qpool.tile([65, S], BF16, tag="qTe", name=f"qTe{h}")
                    nc.gpsimd.memset(qT2[h][64:65, :], 1.0)
                    kT2[h] = kpool.tile([65, S], BF16, tag="kTe", name=f"kTe{h}")
                    nc.gpsimd.dma_start(out=kT2[h][64:65, :], in_=bias8_d[0:1, :])

                for wi, (w_tiles, b_sb, dest) in enumerate(
                        ((w_q, bq2_sb, qT2), (w_k, bk2_sb, kT2))):
                    for sh in range(2):
                        ps = pspool.tile([128, 512], F32, tag="sc",
                                         name=f"qkps{pair}_{sh}")
                        for kt in range(KT):
                            nc.tensor.matmul(
                                ps,
                                lhsT=w_tiles[kt][:, pair * 128:(pair + 1) * 128],
                                rhs=xT[kt][:, sh * 512:(sh + 1) * 512],
                                start=(kt == 0),
                                stop=(kt == KT - 1),
                            )
                        for half in range(2):
                            h = 2 * pair + half
                            dst = dest[h][0:64, sh * 512:(sh + 1) * 512]
                            srcp = ps[half * 64:(half + 1) * 64, :]
                            if skip_bias:
                                if wi == 0:
                                    nc.scalar.copy(dst, srcp)
                                else:
                                    nc.vector.tensor_copy(dst, srcp)
                            elif wi == 0:
                                nc.scalar.activation(
                                    dst, srcp,
                                    mybir.ActivationFunctionType.Identity,
                                    bias=b_sb[half * 64:(half + 1) * 64,
                                              pair:pair + 1],
                                )
                            else:
                                nc.vector.tensor_scalar(
                                    dst, srcp,
                                    b_sb[half * 64:(half + 1) * 64, pair:pair + 1],
                                    None, op0=AluAdd,
                                )

                for h in heads:
                    qT, kT_ = qT2[h], kT2[h]
                    # --- scores^T [k, q] -> single exp -> pT (unnormalized) ---
                    pb = []
                    for kt in range(ST):
                        ps = pspool.tile([128, S], F32, tag="sc", name=f"sB{h}_{kt}")
                        for qh in range(2):
                            nc.tensor.matmul(
                                ps[:, qh * 512:(qh + 1) * 512],
                                lhsT=kT_[:, kt * 128:(kt + 1) * 128],
                                rhs=qT[:, qh * 512:(qh + 1) * 512],
                                start=True,
                                stop=True,
                            )
                        pt = ptpool.tile([128, S], BF16, tag="pt", name=f"pt{h}_{kt}")
                        nc.scalar.activation(pt, ps, Exp, scale=0.125)
                        pb.append(pt)
                    # --- ctx^T (+ sums in row 64 via the v ones column) ---
                    ctxE = []
                    for qh in range(2):
                        pc = cxpool.tile([65, 512], F32, tag="cx",
                                         name=f"ctx{h}_{qh}")
                        for kt in range(ST):
                            nc.tensor.matmul(
                                pc,
                                lhsT=vE[kt][:, h * 65:(h + 1) * 65],
                                rhs=pb[kt][:, qh * 512:(qh + 1) * 512],
                                start=(kt == 0),
                                stop=(kt == ST - 1),
                            )
                        ctxE.append(pc)
                    # copy ctx out unnormalized; frees the psum slots fast
                    half = h % 2
                    for qh in range(2):
                        nc.vector.tensor_copy(
                            ctxT[h // 2][half * 64:(half + 1) * 64,
                                         qh * 512:(qh + 1) * 512],
                            ctxE[qh][0:64, :],
                        )
                    # --- softmax denominators -> reciprocal, broadcast [q] ---
                    srow = srowpool.tile([1, S], F32, tag="srow", name=f"srow{h}")
                    for qh in range(2):
                        nc.vector.tensor_copy(
                            srow[0:1, qh * 512:(qh + 1) * 512], ctxE[qh][64:65, :]
                        )
                    tp = cxpool.tile([128, ST], F32, tag="cx", name=f"tps{h}")
                    for j in range(ST):
                        nc.tensor.transpose(
                            tp[:, j:j + 1],
                            srow[0:1, j * 128:(j + 1) * 128],
                            idn_sb[0:1, 0:1],
                        )
                    recs = recpool.tile([128, ST], F32, tag="recs", name=f"recs{h}")
                    nc.vector.reciprocal(recs, tp)
                    rtr = cxpool.tile([8, 128], F32, tag="cx", name=f"rtr{h}")
                    nc.tensor.transpose(rtr, recs, idn_sb)
                    rT16 = rtpool.tile([8, 128], BF16, tag="rT16", name=f"rT16{h}")
                    nc.vector.tensor_copy(rT16, rtr)
                    nc.sync.dma_start(
                        r16_d[h:h + 1, :].rearrange("a (b c) -> (a b) c", c=128), rT16
                    )
                    rb16 = rb16pool.tile([128, S], BF16, tag="rb16", name=f"rb16{h}")
                    nc.sync.dma_start(rb16, r16_d[h:h + 1, :].to_broadcast([128, S]))
                    # --- ctx normalize in place (sbuf, bf16) ---
                    for qh in range(2):
                        sl = ctxT[h // 2][half * 64:(half + 1) * 64,
                                          qh * 512:(qh + 1) * 512]
                        nc.vector.tensor_mul(
                            sl, sl,
                            rb16[half * 64:(half + 1) * 64,
                                 qh * 512:(qh + 1) * 512],
                        )
                    # --- attn^T normalize + store ---
                    for kt in range(ST):
                        nc.vector.tensor_mul(pb[kt], pb[kt], rb16)
                        nc.sync.dma_start(
                            attn_d[h, kt * 128:(kt + 1) * 128, :], pb[kt]
                        )

            # ============== out-proj + residual + layernorm ===================
            with tc.tile_pool(name="p3x", bufs=2) as x2pool, \
                 tc.tile_pool(name="p3z", bufs=2) as zpool, \
                 tc.tile_pool(name="p3st", bufs=4) as statpool:
                for t in range(ST):
                    x2 = x2pool.tile([128, D], F32, tag="x2", name=f"x2{t}")
                    nc.sync.dma_start(x2, x_d[t * 128:(t + 1) * 128, :])
                    z = zpool.tile([128, D], F32, tag="z", name=f"z{t}")
                    for dh in range(2):
                        ps = pspool.tile([128, 512], F32, tag="sc", name=f"y{t}_{dh}")
                        for ct in range(PAIRS):
                            nc.tensor.matmul(
                                ps,
                                lhsT=ctxT[ct][:, t * 128:(t + 1) * 128],
                                rhs=w_o[ct][:, dh * 512:(dh + 1) * 512],
                                start=(ct == 0),
                                stop=(skip_bo2 and ct == PAIRS - 1),
                            )
                        if not skip_bo2:
                            nc.tensor.matmul(
                                ps,
                                lhsT=ones1,
                                rhs=bo2_sb[0:1, dh * 512:(dh + 1) * 512],
                                start=False,
                                stop=True,
                            )
                        nc.vector.tensor_add(
                            z[:, dh * 512:(dh + 1) * 512],
                            x2[:, dh * 512:(dh + 1) * 512],
                            ps,
                        )
                    stats = statpool.tile([128, 2, 6], F32, tag="stats", name=f"st{t}")
                    for sg in range(2):
                        nc.vector.bn_stats(
                            stats[:, sg, :], z[:, sg * 512:(sg + 1) * 512]
                        )
                    mv = statpool.tile([128, 2], F32, tag="mv", name=f"mv{t}")
                    nc.vector.bn_aggr(mv, stats)
                    sd = statpool.tile([128, 1], F32, tag="sd", name=f"sd{t}")
                    nc.scalar.activation(sd, mv[:, 1:2], SqrtF, bias=eps_sb)
                    rs = statpool.tile([128, 1], F32, tag="rs", name=f"rs{t}")
                    nc.vector.reciprocal(rs, sd)
                    nc.vector.tensor_scalar(
                        z, z, mv[:, 0:1], rs,
                        op0=mybir.AluOpType.subtract,
                        op1=mybir.AluOpType.mult,
                    )
                    if not skip_gamma:
                        nc.vector.tensor_mul(z, z, gamma_b)
                        nc.vector.tensor_add(z, z, beta_b)
                    nc.sync.dma_start(out_d[t * 128:(t + 1) * 128, :], z)

    nc.compile()
    return nc


def _get_nc(skip_bias=False, skip_gamma=False, skip_bo2=False):
    key = ("nc", skip_bias, skip_gamma, skip_bo2)
    if key not in _CACHE:
        _CACHE[key] = _build(skip_bias, skip_gamma, skip_bo2)
    return _CACHE[key]


def _install_ntff_hook():
    """Provide the antenv.axon_hooks shim the boot image lacks, so
    run_bass_kernel_spmd(trace=True) can capture NTFF profiles."""
    try:
        import types

        try:
            from antenv.axon_hooks import get_axon_ntff_profile_hook  # noqa: F401
        except ImportError:
            import antenv

            mod = types.ModuleType("antenv.axon_hooks")
            _hook = [None]
            mod.set_axon_ntff_profile_hook = lambda h: _hook.__setitem__(0, h)
            mod.get_axon_ntff_profile_hook = lambda: _hook[0]
            sys.modules["antenv.axon_hooks"] = mod
            antenv.axon_hooks = mod
        from antenv import axon_hooks

        if axon_hooks.get_axon_ntff_profile_hook() is None:
            from trn_agent_boot.trn_boot import _ntff_profile_via_ctypes

            hook = _ntff_profile_via_ctypes("/opt/axon/libaxon_pjrt.so")
            if hook is None:
                return False
            axon_hooks.set_axon_ntff_profile_hook(hook)
        import concourse.bass_utils as bu

        bu.upload_artifacts = lambda tmpdir: f"local:{tmpdir}"
        return True
    except Exception:
        import traceback

        traceback.print_exc()
        return False


def _to_bf16(a):
    import ml_dtypes

    return np.asarray(a, np.float32).astype(ml_dtypes.bfloat16)


def kernel(x, mask, wq, bq, wk, bk, wv, bv, wo, bo, gamma, beta):
    x = np.asarray(x, np.float32)
    mask = np.asarray(mask)
    wq, wk, wv, wo = (np.asarray(w, np.float32) for w in (wq, wk, wv, wo))
    bq, bk, bv, bo = (np.asarray(b, np.float32) for b in (bq, bk, bv, bo))
    gamma = np.asarray(gamma, np.float32)
    beta = np.asarray(beta, np.float32)

    skip_bias = bool(np.all(bq == 0.0) and np.all(bk == 0.0))
    skip_gamma = bool(np.all(gamma == 1.0) and np.all(beta == 0.0))
    bo2_chk = bo.astype(np.float64) + bv.astype(np.float64) @ wo.astype(np.float64)
    skip_bo2 = bool(np.all(bo2_chk == 0.0))
    nc = _get_nc(skip_bias, skip_gamma, skip_bo2)

    bo2 = (bo.astype(np.float64) + bv.astype(np.float64) @ wo.astype(np.float64))
    bo2 = bo2.astype(np.float32).reshape(1, D)
    shared = {
        "wq": _to_bf16(wq), "wk": _to_bf16(wk),
        "wv": _to_bf16(wv), "wo": _to_bf16(wo),
        "bq2": np.ascontiguousarray(bq.reshape(KT, 128).T),
        "bk2": np.ascontiguousarray(bk.reshape(KT, 128).T),
        "bo2": bo2,
        "gamma_r": gamma.reshape(1, D),
        "beta_r": beta.reshape(1, D),
    }
    in_maps = []
    for c in range(B):
        m = dict(shared)
        m["x"] = np.ascontiguousarray(x[c])
        m["xT"] = np.ascontiguousarray(_to_bf16(x[c].T))
        m["bias8"] = (NINF_BIAS * mask[c].astype(np.float32)).reshape(1, S)
        in_maps.append(m)

    trace = bool(int(os.environ.get("KERNEL_TRACE", "0")))
    if trace:
        trace = _install_ntff_hook()
    try:
        res = run_bass_kernel_spmd(
            nc, in_maps, core_ids=list(range(B)), trace=trace,
        )
    except Exception:
        if not trace:
            raise
        import traceback

        traceback.print_exc()
        res = run_bass_kernel_spmd(
            nc, in_maps, core_ids=list(range(B)), trace=False,
        )
    _CACHE["last_result"] = res

    out = np.stack([np.asarray(res.results[c]["out"], np.float32) for c in range(B)])
    # attn comes back per-head transposed [H, k, q]; swap back to [H, q, k]
    attn = np.empty((B, H, S, S), np.float32)
    for c in range(B):
        attn[c] = np.asarray(res.results[c]["attn"]).astype(np.float32).swapaxes(1, 2)
    return out, attn
